# revision 2
# baseline (speedup 1.0000x reference)
"""MultiHeadSelfAttention + ALiBi for Trainium2, SPMD over 8 NeuronCores.

Sharding: core c handles batch b = c // 4 and head group g = c % 4
(3 of the 12 heads, grouped so per-head ALiBi band sizes balance).
Each core computes y_partial[b] = ctx(heads_g) @ Wout[rows_g]; the host
sums the 4 partials per batch and adds bout.

Device pipeline per core (all fp32):
  1. QK^T = Wqk^T @ x^T  -> per head: Q'/8+bq into dual Q buffers, K+bk
     into K buffer.  V = x @ Wv + bv (ones column appended per head for
     softmax denominators).
  2. S^T blocks [128k x 512q]: matmul with augmented contraction rows
     carrying the attention mask bias and, off-diagonal, the exact ALiBi
     term -slope*|q-k| (linear there).  Diagonal blocks get a fused DVE
     (rel * -slope + S) pass.  exp() on ScalarE over 3-block groups,
     P^T @ V_aug accumulated in PSUM -> unnormalized ctx^T + denom row.
  3. ctx^T = ctx_u^T * (1/denom) (1/x = exp(-ln x) on ScalarE, denom
     broadcast across partitions via a K=1 matmul); y = ctx^T.T @ Wout
     rows.  Blocks where ALiBi decays attention below ~1e-7 relative are
     skipped per the BANDS table (bout is added on the host).
"""

import math
import os

import numpy as np


def _ensure_concourse():
    try:
        import concourse  # noqa: F401
    except ImportError:
        import sys

        for p in ("/opt/trn_rl_repo", "/root/.axon_site/_ro/trn_rl_repo"):
            if os.path.isdir(p) and p not in sys.path:
                sys.path.insert(0, p)


B, L, D, H, DH = 2, 2048, 768, 12, 64
KT = L // 128  # 16 k-tiles
QC = L // 512  # 4 q-chunks
NH = 3  # heads per core
N_CORES = 8
GROUP_SIZE = 3  # exp/S group size in k-tiles (3 PSUM banks)

# Per head-slot key-tile bands per q-chunk (t_lo, t_hi_exclusive).  Slot 0
# holds the wide-band heads (full attention); slots 1/2 hold heads whose
# ALiBi slope decays attention to ~exp(-25) beyond d_max = 36/slope keys,
# so blocks fully outside the band contribute < 1e-7 relative mass.
BANDS = [
    [(0, 16), (0, 16), (0, 16), (0, 16)],  # slot 0: full
    [(0, 9), (0, 13), (3, 16), (7, 16)],  # slot 1: d=576
    [(0, 6), (2, 10), (6, 14), (10, 16)],  # slot 2: d=204
]

# Head groups balanced by ALiBi band size (slopes below): each group gets
# one wide-band, one mid-band and one narrow-band head.
HEAD_GROUPS = [[4, 3, 0], [5, 2, 8], [6, 11, 9], [7, 1, 10]]

NEG_MASK = -1.0e9


def alibi_slopes(n_heads: int) -> np.ndarray:
    def slopes_pow2(n):
        start = 2 ** (-(2 ** -(math.log2(n) - 3)))
        return [start * start**i for i in range(n)]

    if math.log2(n_heads).is_integer():
        s = slopes_pow2(n_heads)
    else:
        cp = 2 ** int(math.floor(math.log2(n_heads)))
        s = slopes_pow2(cp) + slopes_pow2(2 * cp)[0::2][: n_heads - cp]
    return np.asarray(s, dtype=np.float32)


_PROGRAM_CACHE = {}
DEBUG_TAPS = False


def _build_program():
    """Build the (shared, SPMD) Bass program once."""
    if "nc" in _PROGRAM_CACHE:
        return _PROGRAM_CACHE["nc"]

    _ensure_concourse()
    import concourse.mybir as mybir
    import concourse.tile as tile
    from concourse import bacc
    from concourse.bass import ts

    f32 = mybir.dt.float32
    Exp = mybir.ActivationFunctionType.Exp
    MULT = mybir.AluOpType.mult
    ADD = mybir.AluOpType.add

    slopes = alibi_slopes(H)

    nc = bacc.Bacc(None)

    # ---- DRAM I/O ----
    xT_d = nc.dram_tensor("xT", [D, L], f32, kind="ExternalInput")
    wqk_d = nc.dram_tensor("wqk", [D, 2 * DH * NH], f32, kind="ExternalInput")
    bqk_d = nc.dram_tensor("bqk", [128, NH], f32, kind="ExternalInput")
    wv_d = nc.dram_tensor("wv", [D, DH * NH], f32, kind="ExternalInput")
    bv_d = nc.dram_tensor("bv", [1, DH * NH], f32, kind="ExternalInput")
    woutp_d = nc.dram_tensor("woutp", [256, D], f32, kind="ExternalInput")
    augqR_d = nc.dram_tensor("augqR", [3, L], f32, kind="ExternalInput")
    augqL_d = nc.dram_tensor("augqL", [3, L], f32, kind="ExternalInput")
    augk_d = nc.dram_tensor("augk", [NH, 3, L], f32, kind="ExternalInput")
    # rel4[p, j, m, q'] = -slope_j * |q' - 128m - p| (pre-scaled per slot)
    rel4_d = nc.dram_tensor("rel4", [128, NH, 4, 512], f32, kind="ExternalInput")
    y_d = nc.dram_tensor("ypart", [L, D], f32, kind="ExternalOutput")
    if DEBUG_TAPS:
        dbg_q = nc.dram_tensor("dbg_q", [67, L], f32, kind="ExternalOutput")
        dbg_k = nc.dram_tensor("dbg_k", [67, L], f32, kind="ExternalOutput")
        dbg_v = nc.dram_tensor("dbg_v", [128, NH * (DH + 1)], f32, kind="ExternalOutput")
        dbg_pt = nc.dram_tensor("dbg_pt", [128, 3 * 512], f32, kind="ExternalOutput")
        dbg_ou = nc.dram_tensor("dbg_ou", [128, 512], f32, kind="ExternalOutput")
        dbg_rec = nc.dram_tensor("dbg_rec", [64, 512], f32, kind="ExternalOutput")
        dbg_ctx = nc.dram_tensor("dbg_ctx", [128, L], f32, kind="ExternalOutput")

    with tile.TileContext(nc) as tc:
        with tc.tile_pool(name="persist", bufs=1) as pp:
            # ---- persistent SBUF ----
            wqk_sb = pp.tile([128, 6, 2 * DH * NH], f32)
            bqk_sb = pp.tile([128, NH], f32)
            wv_sb = pp.tile([128, 6, DH * NH], f32)
            bv_sb = pp.tile([1, DH * NH], f32)
            woutp_sb = pp.tile([128, 2, D], f32)
            rel4_sb = pp.tile([128, NH, 4, 512], f32)
            ones_sb = pp.tile([1, 128], f32)
            V_sb = pp.tile([128, KT, NH, DH + 1], f32)
            ctxA = pp.tile([128, L], f32)  # heads 0,1 of group
            ctxB = pp.tile([64, L], f32)  # head 2 of group
            # Per-head attention operand buffers.
            # Kbuf rows: 0-63 K^T, 64 mask, 65 s*k_idx, 66 s*1
            # QbufR rows: 0-63 Q'^T, 64 1, 65 1, 66 -q_idx
            # QbufL rows: 0-63 Q'^T, 64 1, 65 -1, 66 +q_idx
            Kbuf = [
                pp.tile([67, L], f32, tag=f"kb{j}", name=f"kb{j}") for j in range(NH)
            ]
            QbufR = [
                pp.tile([67, L], f32, tag=f"qr{j}", name=f"qr{j}") for j in range(NH)
            ]
            QbufL = [
                pp.tile([67, L], f32, tag=f"ql{j}", name=f"ql{j}") for j in range(NH)
            ]

            nc.sync.dma_start(wqk_sb[:], wqk_d.rearrange("(o p) m -> p o m", p=128))
            nc.sync.dma_start(bqk_sb[:], bqk_d[:])
            nc.sync.dma_start(wv_sb[:], wv_d.rearrange("(o p) m -> p o m", p=128))
            nc.sync.dma_start(bv_sb[:], bv_d[:])
            nc.sync.dma_start(woutp_sb[:], woutp_d.rearrange("(o p) n -> p o n", p=128))
            nc.sync.dma_start(rel4_sb[:], rel4_d[:])
            nc.vector.memset(ones_sb[:], 1.0)
            nc.gpsimd.memset(V_sb[:, :, :, DH : DH + 1], 1.0)
            for j in range(NH):
                nc.sync.dma_start(QbufR[j][64:67, :], augqR_d[:])
                nc.sync.dma_start(QbufL[j][64:67, :], augqL_d[:])
                nc.sync.dma_start(Kbuf[j][64:67, :], augk_d[j])

            # Pre-touch DMA-loaded tiles with their consuming engines so
            # later TensorScalarPtr ops carry a single sync wait (the
            # walrus TS encoding rejects multi-wait instructions).
            junk = pp.tile([1, 16], f32, name="junk")
            nc.vector.tensor_copy(junk[0:1, 0:1], bqk_sb[0:1, 0:1])
            nc.vector.tensor_copy(junk[0:1, 1:2], rel4_sb[0:1, 0, 0, 0:1])
            for j in range(NH):
                nc.vector.tensor_copy(junk[0:1, 2 + j : 3 + j], QbufR[j][64:65, 0:1])
                nc.vector.tensor_copy(junk[0:1, 5 + j : 6 + j], QbufL[j][64:65, 0:1])
                nc.vector.tensor_copy(junk[0:1, 8 + j : 9 + j], Kbuf[j][64:65, 0:1])
            junk2 = pp.tile([1, 4], f32, name="junk2")
            nc.scalar.copy(junk2[0:1, 0:1], V_sb[0:1, 0, 0, DH : DH + 1])

            # ---- stage 1: QKV projection ----
            with (
                tc.tile_pool(name="xpool", bufs=1) as xp,
                tc.tile_pool(name="ps1", bufs=4, space="PSUM") as ps1,
            ):
                xT_sb = xp.tile([128, 6, L], f32)
                for kt in range(6):
                    nc.sync.dma_start(
                        xT_sb[:, kt, :],
                        xT_d.rearrange("(o p) f -> p o f", p=128)[:, kt, :],
                    )

                # Q^T/K^T per head: PSUM [128, 512] = [Q^T_h; K^T_h] chunk
                for j in range(NH):
                    pcs = [
                        ps1.tile([128, 512], f32, tag="ps1", name=f"ps1c{c}")
                        for c in range(QC)
                    ]
                    for kt in range(6):
                        for c in range(QC):
                            nc.tensor.matmul(
                                pcs[c],
                                wqk_sb[:, kt, ts(j, 128)],
                                xT_sb[:, kt, ts(c, 512)],
                                start=(kt == 0),
                                stop=(kt == 5),
                            )
                    for c in range(QC):
                        ps = pcs[c]
                        cs = ts(c, 512)
                        nc.vector.tensor_scalar(
                            QbufR[j][0:64, cs],
                            ps[0:64, :],
                            0.125,
                            bqk_sb[0:64, j : j + 1],
                            MULT,
                            ADD,
                        )
                        nc.vector.tensor_copy(QbufL[j][0:64, cs], QbufR[j][0:64, cs])
                        nc.vector.tensor_scalar(
                            Kbuf[j][0:64, cs],
                            ps[64:128, :],
                            bqk_sb[64:128, j : j + 1],
                            None,
                            ADD,
                        )

                # V natural layout [l, d] + bias via K=1 matmul
                for lt in range(KT):
                    psv = ps1.tile([128, 512], f32, tag="ps1", name="psv")[:, : DH * NH]
                    for kt in range(6):
                        nc.tensor.matmul(
                            psv,
                            xT_sb[:, kt, ts(lt, 128)],
                            wv_sb[:, kt, :],
                            start=(kt == 0),
                            stop=False,
                        )
                    nc.tensor.matmul(
                        psv,
                        ones_sb[0:1, 0:128],
                        bv_sb[0:1, :],
                        start=False,
                        stop=True,
                    )
                    nc.scalar.copy(
                        V_sb[:, lt, :, 0:DH],
                        psv.rearrange("p (h x) -> p h x", x=DH),
                    )

            # ---- stage 2: attention ----
            with (
                tc.tile_pool(name="psS", bufs=2, space="PSUM") as psS,
                tc.tile_pool(name="psO", bufs=2, space="PSUM") as psO,
                tc.tile_pool(name="ptp", bufs=3) as ptp,
                tc.tile_pool(name="nrm", bufs=2) as nrm,
            ):
                for j in range(NH):
                    for c in range(QC):
                        cs = ts(c, 512)
                        t_lo, t_hi = BANDS[j][c]
                        out_t = psO.tile([128, 512], f32, tag="outaug")
                        for t0 in range(t_lo, t_hi, GROUP_SIZE):
                            tn = min(GROUP_SIZE, t_hi - t0)
                            st = psS.tile([128, GROUP_SIZE * 512], f32, tag="st")
                            for i in range(tn):
                                t = t0 + i
                                js = ts(i, 512)
                                m = t - 4 * c
                                if 0 <= m < 4:  # diagonal block
                                    nc.tensor.matmul(
                                        st[:, js],
                                        Kbuf[j][0:65, ts(t, 128)],
                                        QbufR[j][0:65, cs],
                                        start=True,
                                        stop=True,
                                    )
                                    nc.vector.scalar_tensor_tensor(
                                        st[:, js],
                                        rel4_sb[:, j, m, :],
                                        1.0,
                                        st[:, js],
                                        MULT,
                                        ADD,
                                    )
                                elif c > t // 4:  # strictly right of diag
                                    nc.tensor.matmul(
                                        st[:, js],
                                        Kbuf[j][0:67, ts(t, 128)],
                                        QbufR[j][0:67, cs],
                                        start=True,
                                        stop=True,
                                    )
                                else:  # strictly left
                                    nc.tensor.matmul(
                                        st[:, js],
                                        Kbuf[j][0:67, ts(t, 128)],
                                        QbufL[j][0:67, cs],
                                        start=True,
                                        stop=True,
                                    )
                            pt = ptp.tile([128, GROUP_SIZE * 512], f32, tag="pt")
                            nc.scalar.activation(
                                pt[:, : tn * 512], st[:, : tn * 512], Exp
                            )
                            if DEBUG_TAPS and j == 0 and c == 0 and g == 0:
                                nc.sync.dma_start(dbg_pt[:], pt[:])
                            for i in range(tn):
                                t = t0 + i
                                nc.tensor.matmul(
                                    out_t[0 : DH + 1, :],
                                    V_sb[:, t, j, :],
                                    pt[:, ts(i, 512)],
                                    start=(t == t_lo),
                                    stop=(t == t_hi - 1),
                                    skip_group_check=True,
                                )
                        # 1/denom = exp(-ln(denom)) on ScalarE (the custom
                        # DVE reciprocal ops misbehave under this runtime).
                        lnr = nrm.tile([1, 512], f32, tag="lnr")
                        nc.scalar.activation(
                            lnr, out_t[DH : DH + 1, :], mybir.ActivationFunctionType.Ln
                        )
                        rec = nrm.tile([1, 512], f32, tag="rec")
                        nc.scalar.activation(rec, lnr, Exp, scale=-1.0)
                        # broadcast 1/denom across 64 partitions via K=1
                        # matmul into a base-0 PSUM tile (partition-offset
                        # matmul outputs misbehave on HW), then copy out.
                        recb_ps = psS.tile(
                            [128, GROUP_SIZE * 512], f32, tag="st", name="recps"
                        )[0:64, 0:512]
                        nc.tensor.matmul(
                            recb_ps,
                            ones_sb[0:1, 0:64],
                            rec,
                            start=True,
                            stop=True,
                            skip_group_check=True,
                        )
                        recb = nrm.tile([64, 512], f32, tag="recb")
                        nc.vector.tensor_copy(recb, recb_ps)
                        if j < 2:
                            ctx_slice = ctxA[j * 64 : (j + 1) * 64, cs]
                        else:
                            ctx_slice = ctxB[0:64, cs]
                        if DEBUG_TAPS and j == 0 and c == 0:
                            ou_sb = nrm.tile([128, 512], f32, tag="ousb", name="ousb")
                            nc.vector.tensor_copy(ou_sb[0:64, :], out_t[0:64, :])
                            nc.vector.tensor_copy(ou_sb[64:128, :], out_t[64:128, :])
                            nc.sync.dma_start(dbg_ou[:], ou_sb[:])
                            nc.sync.dma_start(dbg_rec[:], recb[:])
                        nc.vector.tensor_mul(ctx_slice, out_t[0:DH, :], recb)

            if DEBUG_TAPS:
                nc.sync.dma_start(dbg_q[:], QbufR[0][:])
                nc.sync.dma_start(dbg_k[:], Kbuf[0][:])
                nc.sync.dma_start(dbg_v[:], V_sb[:, 0, :, :].rearrange("p h x -> p (h x)"))
                nc.sync.dma_start(dbg_ctx[:], ctxA[:])

            # ---- stage 3: output projection ----
            with (
                tc.tile_pool(name="ps3", bufs=2, space="PSUM") as ps3,
                tc.tile_pool(name="ysb", bufs=3) as yp,
            ):
                for lt in range(KT):
                    y = yp.tile([128, D], f32, tag="y")
                    for n0, nw in ((0, 512), (512, 256)):
                        ps = ps3.tile([128, 512], f32, tag="ps3", name="ps3t")[:, :nw]
                        nc.tensor.matmul(
                            ps,
                            ctxA[:, ts(lt, 128)],
                            woutp_sb[:, 0, n0 : n0 + nw],
                            start=True,
                            stop=False,
                        )
                        nc.tensor.matmul(
                            ps,
                            ctxB[0:64, ts(lt, 128)],
                            woutp_sb[0:64, 1, n0 : n0 + nw],
                            start=False,
                            stop=True,
                        )
                        nc.scalar.copy(y[:, n0 : n0 + nw], ps)
                    nc.sync.dma_start(y_d[ts(lt, 128), :], y)

    if not nc.is_finalized():
        nc.finalize()
    _PROGRAM_CACHE["nc"] = nc
    return nc


def _host_inputs(x, attn_mask, Wqkv, bqkv, Wout, bout):
    """Build the 8 per-core input dicts."""
    slopes = alibi_slopes(H)
    x = np.asarray(x, dtype=np.float32)
    attn_mask = np.asarray(attn_mask)
    Wqkv = np.asarray(Wqkv, dtype=np.float32)
    bqkv = np.asarray(bqkv, dtype=np.float32)
    Wout = np.asarray(Wout, dtype=np.float32)
    bout = np.asarray(bout, dtype=np.float32)

    q_idx = np.arange(L, dtype=np.float32)
    ones_row = np.ones(L, dtype=np.float32)
    augqR = np.ascontiguousarray(np.stack([ones_row, ones_row, -q_idx]))
    augqL = np.ascontiguousarray(np.stack([ones_row, -ones_row, q_idx]))

    # rel4[p, m, q'] = |q' - 128m - p|  (diagonal-block relative distance)
    p = np.arange(128, dtype=np.float32)[:, None, None]
    m = np.arange(4, dtype=np.float32)[None, :, None]
    qq = np.arange(512, dtype=np.float32)[None, None, :]
    rel4_base = np.abs(qq - 128.0 * m - p).astype(np.float32)

    in_maps = []
    for core in range(N_CORES):
        b = core // 4
        g = core % 4
        heads = HEAD_GROUPS[g]

        wqk = np.empty((D, 2 * DH * NH), np.float32)
        bqk = np.empty((128, NH), np.float32)
        wv = np.empty((D, DH * NH), np.float32)
        bv = np.empty((1, DH * NH), np.float32)
        woutp = np.zeros((256, D), np.float32)
        augk = np.empty((NH, 3, L), np.float32)
        mask_row = np.where(attn_mask[b] == 0, NEG_MASK, 0.0).astype(np.float32)
        rel4 = np.empty((128, NH, 4, 512), np.float32)
        for jj, h in enumerate(heads):
            rel4[:, jj] = -float(slopes[h]) * rel4_base
            wqk[:, jj * 128 : jj * 128 + 64] = Wqkv[:, h * DH : (h + 1) * DH]
            wqk[:, jj * 128 + 64 : (jj + 1) * 128] = Wqkv[
                :, D + h * DH : D + (h + 1) * DH
            ]
            bqk[0:64, jj] = bqkv[h * DH : (h + 1) * DH] * 0.125
            bqk[64:128, jj] = bqkv[D + h * DH : D + (h + 1) * DH]
            wv[:, jj * DH : (jj + 1) * DH] = Wqkv[:, 2 * D + h * DH : 2 * D + (h + 1) * DH]
            bv[0, jj * DH : (jj + 1) * DH] = bqkv[2 * D + h * DH : 2 * D + (h + 1) * DH]
            woutp[jj * DH : (jj + 1) * DH, :] = Wout[h * DH : (h + 1) * DH, :]
            s = float(slopes[h])
            augk[jj, 0, :] = mask_row
            augk[jj, 1, :] = s * q_idx  # s * k_idx along keys
            augk[jj, 2, :] = s
        in_maps.append(
            {
                "xT": np.ascontiguousarray(x[b].T),
                "wqk": wqk,
                "bqk": bqk,
                "wv": wv,
                "bv": bv,
                "woutp": woutp,
                "augqR": augqR,
                "augqL": augqL,
                "augk": augk,
                "rel4": rel4,
            }
        )
    return in_maps


def kernel(x, attn_mask, Wqkv, bqkv, Wout, bout):
    _ensure_concourse()
    from concourse.bass_utils import run_bass_kernel_spmd

    nc = _build_program()
    in_maps = _host_inputs(x, attn_mask, Wqkv, bqkv, Wout, bout)
    for m in in_maps:
        m.pop("ypart", None)

    res = run_bass_kernel_spmd(
        nc,
        in_maps,
        list(range(N_CORES)),
        trace=bool(os.environ.get("BASS_TRACE")),
    )
    outs = [r["ypart"] for r in res.results]
    out = np.zeros((B, L, D), np.float32)
    for core in range(N_CORES):
        out[core // 4] += outs[core]
    out += np.asarray(bout, np.float32)[None, None, :]
    kernel.last_result = res
    if res.exec_time_ns is not None:
        kernel.last_exec_time_ns = res.exec_time_ns
    return out



# revision 28
# speedup vs baseline: 2.0878x; 2.0878x over previous
"""MultiHeadSelfAttention + ALiBi for Trainium2, SPMD over 8 NeuronCores.

Sharding: core c handles batch b = c // 4 and head group g = c % 4
(3 of the 12 heads, grouped so per-head ALiBi band sizes balance).
Each core computes y_partial[b] = ctx(heads_g) @ Wout[rows_g]; the host
sums the 4 partials per batch and adds bout.

All matmuls run in float32r (12-mantissa-bit fast mode, 4x the fp32
rate at free-dim >= 256).  f32r matmul operands must come from rounding
producers; DMA qualifies when the DRAM tensor is declared f32r, so the
host pre-rounds x/weights/aug rows to the 12-bit grid and the kernel
DMAs them straight into f32r tiles.  Device-computed operands (Q/K/V,
P=exp(S), ctx) round via their producing DVE/ACT instruction.

ALiBi handling under reduced precision: the per-head slope is
pre-rounded to 10 mantissa bits (s_r) and used consistently on both the
k and q aug rows, making slope rounding a per-head slope perturbation
(rel ~5e-4, harmless) instead of a softmax distortion.  The s_r*k_idx
aug row splits into v = round12(s_r*k) plus a residual row
r = s_r*k - v so large magnitudes survive the 12-bit grid exactly.

Device pipeline per core:
  1. QK^T = Wqk^T @ x^T  -> per head: Q'/8+bq into dual Q buffers, K+bk
     into K buffer.  V = x @ Wv + bv (ones column appended per head for
     softmax denominators).
  2. S^T blocks [128k x 512q]: matmul with augmented contraction rows
     carrying the attention mask bias and, off-diagonal, the ALiBi term
     -s_r*|q-k| (linear there).  Diagonal blocks get a fused DVE
     (rel_base * -s_r + S) pass.  exp() on ScalarE over 3-block groups,
     P^T @ V_aug accumulated in PSUM -> unnormalized ctx^T + denom row,
     copied to SBUF per (head, chunk).
  3. Batched normalization: one Ln over all 12 denom rows, then per-pair
     Exp(-x) (exp table loads once), K=1 broadcast matmul + in-place DVE
     multiply.  1/x = exp(-ln x) because the DVE reciprocal ops
     misbehave under this runtime.
  4. y = ctx^T.T @ Wout rows.  Blocks where ALiBi decays attention below
     ~2e-4 relative are skipped per the BANDS table (bout on the host).
"""

import math
import os

import numpy as np


def _ensure_concourse():
    try:
        import concourse  # noqa: F401
    except ImportError:
        import sys

        for p in ("/opt/trn_rl_repo", "/root/.axon_site/_ro/trn_rl_repo"):
            if os.path.isdir(p) and p not in sys.path:
                sys.path.insert(0, p)


B, L, D, H, DH = 2, 2048, 768, 12, 64
KT = L // 128  # 16 k-tiles
QC = L // 512  # 4 q-chunks
NH = 3  # heads per core
N_CORES = 8
GROUP_SIZE = 3  # exp/S group size in k-tiles (3 PSUM banks)

# Per head-slot key-tile bands per q-chunk (t_lo, t_hi_exclusive).  Slot
# 0 holds the wide-band heads (full attention); slots 1/2 hold heads
# whose ALiBi decays attention to ~exp(-36) beyond d_max = 36/slope
# keys.  The margin must cover worst-case Q.K swings (~±5 on each side
# of the exponent), so excluded blocks leak < ~2048*e^(-26) ~ 1e-8.
BANDS = [
    [(0, 16), (0, 16), (0, 16), (0, 16)],  # slot 0: full
    [(0, 9), (0, 13), (3, 16), (7, 16)],  # slot 1: d=576
    [(0, 6), (2, 10), (6, 14), (10, 16)],  # slot 2: d=204
]

# Head groups balanced by ALiBi band size (slopes below): each group gets
# one wide-band, one mid-band and one narrow-band head.
HEAD_GROUPS = [[4, 3, 0], [5, 2, 8], [6, 11, 9], [7, 1, 10]]

NEG_MASK = -1.0e9


def alibi_slopes(n_heads: int) -> np.ndarray:
    def slopes_pow2(n):
        start = 2 ** (-(2 ** -(math.log2(n) - 3)))
        return [start * start**i for i in range(n)]

    if math.log2(n_heads).is_integer():
        s = slopes_pow2(n_heads)
    else:
        cp = 2 ** int(math.floor(math.log2(n_heads)))
        s = slopes_pow2(cp) + slopes_pow2(2 * cp)[0::2][: n_heads - cp]
    return np.asarray(s, dtype=np.float32)


def _round_mant(x, bits):
    """Round fp32 values to `bits` explicit mantissa bits (RNE), i.e. onto
    the f32r grid (12) or safely within it (10)."""
    x = np.asarray(x, np.float32)
    b = x.view(np.uint32).copy()
    drop = 23 - bits
    b = b + (((b >> drop) & 1) + np.uint32((1 << (drop - 1)) - 1))
    b &= np.uint32(~((1 << drop) - 1) & 0xFFFFFFFF)
    return b.view(np.float32)


def _round10(x):
    return _round_mant(x, 10)


def _round12(x):
    return _round_mant(x, 12)


_PROGRAM_CACHE = {}


def _build_program():
    """Build the (shared, SPMD) Bass program once."""
    if "nc" in _PROGRAM_CACHE:
        return _PROGRAM_CACHE["nc"]

    _ensure_concourse()
    import concourse.mybir as mybir
    import concourse.tile as tile
    from concourse import bacc
    from concourse.bass import ts

    f32 = mybir.dt.float32
    f32r = mybir.dt.float32r
    Exp = mybir.ActivationFunctionType.Exp
    Ln = mybir.ActivationFunctionType.Ln
    MULT = mybir.AluOpType.mult
    ADD = mybir.AluOpType.add

    nc = bacc.Bacc(None)

    # ---- DRAM I/O (f32r tensors arrive pre-rounded from the host) ----
    xT_d = nc.dram_tensor("xT", [D, L], f32r, kind="ExternalInput")
    wqk_d = nc.dram_tensor("wqk", [D, 2 * DH * NH], f32r, kind="ExternalInput")
    bqk_d = nc.dram_tensor("bqk", [128, NH], f32, kind="ExternalInput")
    wv_d = nc.dram_tensor("wv", [D, DH * NH], f32r, kind="ExternalInput")
    bv_d = nc.dram_tensor("bv", [1, DH * NH], f32r, kind="ExternalInput")
    woutp_d = nc.dram_tensor("woutp", [256, D], f32r, kind="ExternalInput")
    ones_d = nc.dram_tensor("onesr", [1, 128], f32r, kind="ExternalInput")
    augqR_d = nc.dram_tensor("augqR", [NH, 5, L], f32r, kind="ExternalInput")
    augqL_d = nc.dram_tensor("augqL", [NH, 5, L], f32r, kind="ExternalInput")
    augk_d = nc.dram_tensor("augk", [NH, 5, L], f32r, kind="ExternalInput")
    # rel_base[p, m, q'] = |q' - 128m - p| (unscaled; -s_r applied on DVE)
    relb_d = nc.dram_tensor("relb", [128, 4, 512], f32, kind="ExternalInput")
    # negslope[p, j] = -s_r of the core's head slot j (per-partition bcast)
    nslp_d = nc.dram_tensor("nslp", [128, NH], f32, kind="ExternalInput")
    y_d = nc.dram_tensor("ypart", [L, D], f32, kind="ExternalOutput")

    with tile.TileContext(nc) as tc:
        with tc.tile_pool(name="persist", bufs=1) as pp:
            # ---- persistent SBUF ----
            bqk_sb = pp.tile([128, NH], f32)
            nslp_sb = pp.tile([128, NH], f32)
            woutp_sb = pp.tile([128, 2, D], f32r)
            relb_sb = pp.tile([128, 4, 512], f32)
            ones_sb = pp.tile([1, 128], f32r)
            onesv_f = pp.tile([128, KT * NH], f32)
            V_sb = pp.tile([128, KT, NH, DH + 1], f32r)
            ctxA = pp.tile([128, L], f32r)  # heads 0,1 of group
            ctxB = pp.tile([64, L], f32r)  # head 2 of group
            # Per-head attention operand buffers.  The f32r PE rounds each
            # product to ~12 significand bits, so every aug product must be
            # a pre-rounded value times +-1: slope*idx terms are
            # premultiplied on the host (v/rv on the k side, rq/rr on the
            # q side, each split into a 12-sig-bit value plus residual).
            # Kbuf rows: 0-63 K^T, 64 mask, 65 v=rnd(s_r*k), 66 rv, 67 1, 68 1
            # QbufR rows: 0-63 Q'^T, 64 1, 65  1, 66  1, 67 -rq, 68 -rr
            # QbufL rows: 0-63 Q'^T, 64 1, 65 -1, 66 -1, 68  rq, 68  rr
            Kbuf = [
                pp.tile([69, L], f32r, tag=f"kb{j}", name=f"kb{j}") for j in range(NH)
            ]
            QbufR = [
                pp.tile([69, L], f32r, tag=f"qr{j}", name=f"qr{j}") for j in range(NH)
            ]
            QbufL = [
                pp.tile([69, L], f32r, tag=f"ql{j}", name=f"ql{j}") for j in range(NH)
            ]

            nc.sync.dma_start(bqk_sb[:], bqk_d[:])
            nc.sync.dma_start(nslp_sb[:], nslp_d[:])
            nc.sync.dma_start(woutp_sb[:], woutp_d.rearrange("(o p) n -> p o n", p=128))
            nc.sync.dma_start(relb_sb[:], relb_d[:])
            nc.sync.dma_start(ones_sb[:], ones_d[:])
            nc.gpsimd.memset(onesv_f[:], 1.0)
            # V softmax-denominator ones column (DVE copy rounds to f32r)
            nc.vector.tensor_copy(
                V_sb[:, :, :, DH : DH + 1].rearrange("p t h o -> p (t h o)"),
                onesv_f[:],
            )
            for j in range(NH):
                nc.sync.dma_start(QbufR[j][64:69, :], augqR_d[j])
                nc.sync.dma_start(QbufL[j][64:69, :], augqL_d[j])
                nc.sync.dma_start(Kbuf[j][64:69, :], augk_d[j])

            # Pre-touch DMA-loaded tiles consumed by TensorScalarPtr ops so
            # those ops carry a single sync wait (the walrus TS encoding
            # rejects multi-wait instructions).
            junk = pp.tile([1, 4], f32, name="junk")
            nc.vector.tensor_copy(junk[0:1, 0:1], bqk_sb[0:1, 0:1])
            nc.vector.tensor_copy(junk[0:1, 1:2], relb_sb[0:1, 0, 0:1])
            nc.vector.tensor_copy(junk[0:1, 2:3], nslp_sb[0:1, 0:1])

            # ---- stage 1: QKV projection ----
            with (
                tc.tile_pool(name="xpool", bufs=1) as xp,
                tc.tile_pool(name="ps1", bufs=4, space="PSUM") as ps1,
            ):
                wqk_sb = xp.tile([128, 6, 2 * DH * NH], f32r)
                wv_sb = xp.tile([128, 6, DH * NH], f32r)
                bv_sb = xp.tile([1, DH * NH], f32r)
                nc.sync.dma_start(wqk_sb[:], wqk_d.rearrange("(o p) m -> p o m", p=128))
                nc.sync.dma_start(wv_sb[:], wv_d.rearrange("(o p) m -> p o m", p=128))
                nc.sync.dma_start(bv_sb[:], bv_d[:])
                xT_sb = xp.tile([128, 6, L], f32r)
                for kt in range(6):
                    nc.sync.dma_start(
                        xT_sb[:, kt, :],
                        xT_d.rearrange("(o p) f -> p o f", p=128)[:, kt, :],
                    )

                # Q^T/K^T per head: PSUM [128, 512] = [Q^T_h; K^T_h] chunk
                for j in range(NH):
                    pcs = [
                        ps1.tile([128, 512], f32, tag="ps1", name=f"ps1c{c}")
                        for c in range(QC)
                    ]
                    for kt in range(6):
                        for c in range(QC):
                            nc.tensor.matmul(
                                pcs[c],
                                wqk_sb[:, kt, ts(j, 128)],
                                xT_sb[:, kt, ts(c, 512)],
                                start=(kt == 0),
                                stop=(kt == 5),
                            )
                    for c in range(QC):
                        ps = pcs[c]
                        cs = ts(c, 512)
                        nc.vector.tensor_scalar(
                            QbufR[j][0:64, cs],
                            ps[0:64, :],
                            0.125,
                            bqk_sb[0:64, j : j + 1],
                            MULT,
                            ADD,
                        )
                        nc.vector.tensor_copy(QbufL[j][0:64, cs], QbufR[j][0:64, cs])
                        nc.vector.tensor_scalar(
                            Kbuf[j][0:64, cs],
                            ps[64:128, :],
                            bqk_sb[64:128, j : j + 1],
                            None,
                            ADD,
                        )

                # V natural layout [l, d] + bias via K=1 matmul
                for lt in range(KT):
                    psv = ps1.tile([128, 512], f32, tag="ps1", name="psv")[:, : DH * NH]
                    for kt in range(6):
                        nc.tensor.matmul(
                            psv,
                            xT_sb[:, kt, ts(lt, 128)],
                            wv_sb[:, kt, :],
                            start=(kt == 0),
                            stop=False,
                        )
                    nc.tensor.matmul(
                        psv,
                        ones_sb[0:1, 0:128],
                        bv_sb[0:1, :],
                        start=False,
                        stop=True,
                    )
                    nc.scalar.copy(
                        V_sb[:, lt, :, 0:DH],
                        psv.rearrange("p (h x) -> p h x", x=DH),
                    )

            # ---- stage 2: attention ----
            # [1, 12*512] denom layout keeps every slice at partition base
            # 0 (activations reject input bases outside 0/32/64/96); the
            # pool wraps stages 2+2b and closes before stage 3.
            with tc.tile_pool(name="normp", bufs=1) as npool:
                denom_sb = npool.tile([1, NH * QC * 512], f32)
                lnr_sb = npool.tile([1, NH * QC * 512], f32)
                with (
                    tc.tile_pool(name="psS", bufs=2, space="PSUM") as psS,
                    tc.tile_pool(name="psO", bufs=2, space="PSUM") as psO,
                    tc.tile_pool(name="ptp", bufs=3) as ptp,
                ):
                  for j in range(NH):
                    for c in range(QC):
                        cs = ts(c, 512)
                        t_lo, t_hi = BANDS[j][c]
                        out_t = psO.tile([128, 512], f32, tag="outaug")
                        for t0 in range(t_lo, t_hi, GROUP_SIZE):
                            tn = min(GROUP_SIZE, t_hi - t0)
                            st = psS.tile([128, GROUP_SIZE * 512], f32, tag="st")
                            for i in range(tn):
                                t = t0 + i
                                js = ts(i, 512)
                                m = t - 4 * c
                                if 0 <= m < 4:  # diagonal block
                                    nc.tensor.matmul(
                                        st[:, js],
                                        Kbuf[j][0:65, ts(t, 128)],
                                        QbufR[j][0:65, cs],
                                        start=True,
                                        stop=True,
                                    )
                                    nc.vector.scalar_tensor_tensor(
                                        st[:, js],
                                        relb_sb[:, m, :],
                                        nslp_sb[:, j : j + 1],
                                        st[:, js],
                                        MULT,
                                        ADD,
                                    )
                                elif c > t // 4:  # keys before queries
                                    nc.tensor.matmul(
                                        st[:, js],
                                        Kbuf[j][0:69, ts(t, 128)],
                                        QbufR[j][0:69, cs],
                                        start=True,
                                        stop=True,
                                    )
                                else:  # keys after queries
                                    nc.tensor.matmul(
                                        st[:, js],
                                        Kbuf[j][0:69, ts(t, 128)],
                                        QbufL[j][0:69, cs],
                                        start=True,
                                        stop=True,
                                    )
                            pt = ptp.tile([128, GROUP_SIZE * 512], f32r, tag="pt")
                            nc.scalar.activation(
                                pt[:, : tn * 512], st[:, : tn * 512], Exp
                            )
                            for i in range(tn):
                                t = t0 + i
                                nc.tensor.matmul(
                                    out_t[0 : DH + 1, :],
                                    V_sb[:, t, j, :],
                                    pt[:, ts(i, 512)],
                                    start=(t == t_lo),
                                    stop=(t == t_hi - 1),
                                    skip_group_check=True,
                                )
                        # stash unnormalized ctx + denom; normalize later
                        if j < 2:
                            ctx_slice = ctxA[j * 64 : (j + 1) * 64, cs]
                        else:
                            ctx_slice = ctxB[0:64, cs]
                        nc.scalar.copy(ctx_slice, out_t[0:DH, :])
                        nc.scalar.copy(
                            denom_sb[0:1, ts(j * QC + c, 512)],
                            out_t[DH : DH + 1, :],
                        )

                # ---- stage 2b: batched normalization ----
                # 1/denom = exp(-ln(denom)); one Ln batch then per-pair
                # Exp (table loads once each), K=1 broadcast matmul,
                # in-place mul.
                with (
                    tc.tile_pool(name="psN", bufs=4, space="PSUM") as psN,
                    tc.tile_pool(name="recp", bufs=2) as rp,
                ):
                    nc.scalar.activation(lnr_sb[:], denom_sb[:], Ln)
                    for j in range(NH):
                        for c in range(QC):
                            cs = ts(c, 512)
                            row = j * QC + c
                            # per-pair Exp lands at base partition 0 for
                            # the K=1 broadcast matmul (base 0/32/64)
                            rec = rp.tile([1, 512], f32r, tag="rec")
                            nc.scalar.activation(
                                rec, lnr_sb[0:1, ts(row, 512)], Exp, scale=-1.0
                            )
                            recb = psN.tile([64, 512], f32, tag="recb")
                            nc.tensor.matmul(
                                recb,
                                ones_sb[0:1, 0:64],
                                rec,
                                start=True,
                                stop=True,
                            )
                            if j < 2:
                                ctx_slice = ctxA[j * 64 : (j + 1) * 64, cs]
                            else:
                                ctx_slice = ctxB[0:64, cs]
                            nc.vector.tensor_mul(ctx_slice, ctx_slice, recb)

            # ---- stage 3: output projection ----
            with (
                tc.tile_pool(name="ps3", bufs=2, space="PSUM") as ps3,
                tc.tile_pool(name="ysb", bufs=3) as yp,
            ):
                for lt in range(KT):
                    y = yp.tile([128, D], f32, tag="y")
                    for n0, nw in ((0, 512), (512, 256)):
                        ps = ps3.tile([128, 512], f32, tag="ps3", name="ps3t")[:, :nw]
                        nc.tensor.matmul(
                            ps,
                            ctxA[:, ts(lt, 128)],
                            woutp_sb[:, 0, n0 : n0 + nw],
                            start=True,
                            stop=False,
                        )
                        nc.tensor.matmul(
                            ps,
                            ctxB[0:64, ts(lt, 128)],
                            woutp_sb[0:64, 1, n0 : n0 + nw],
                            start=False,
                            stop=True,
                        )
                        nc.scalar.copy(y[:, n0 : n0 + nw], ps)
                    nc.sync.dma_start(y_d[ts(lt, 128), :], y)

    if not nc.is_finalized():
        nc.finalize()
    _PROGRAM_CACHE["nc"] = nc
    return nc


def _host_inputs(x, attn_mask, Wqkv, bqkv, Wout, bout):
    """Build the 8 per-core input dicts (f32r operands pre-rounded)."""
    slopes_r = _round10(alibi_slopes(H))
    x = np.asarray(x, dtype=np.float32)
    attn_mask = np.asarray(attn_mask)
    Wqkv = _round12(np.asarray(Wqkv, dtype=np.float32))
    bqkv = np.asarray(bqkv, dtype=np.float32)
    Wout = _round12(np.asarray(Wout, dtype=np.float32))
    bout = np.asarray(bout, dtype=np.float32)

    q_idx = np.arange(L, dtype=np.float32)
    ones_row = np.ones(L, dtype=np.float32)

    # rel_base[p, m, q'] = |q' - 128m - p|  (diagonal-block distance)
    p = np.arange(128, dtype=np.float32)[:, None, None]
    m = np.arange(4, dtype=np.float32)[None, :, None]
    qq = np.arange(512, dtype=np.float32)[None, None, :]
    relb = np.abs(qq - 128.0 * m - p).astype(np.float32)

    onesr = np.ones((1, 128), np.float32)

    in_maps = []
    for core in range(N_CORES):
        b = core // 4
        g = core % 4
        heads = HEAD_GROUPS[g]

        wqk = np.empty((D, 2 * DH * NH), np.float32)
        bqk = np.empty((128, NH), np.float32)
        wv = np.empty((D, DH * NH), np.float32)
        bv = np.empty((1, DH * NH), np.float32)
        woutp = np.zeros((256, D), np.float32)
        augk = np.empty((NH, 5, L), np.float32)
        augqR = np.empty((NH, 5, L), np.float32)
        augqL = np.empty((NH, 5, L), np.float32)
        nslp = np.empty((128, NH), np.float32)
        mask_row = np.where(attn_mask[b] == 0, NEG_MASK, 0.0).astype(np.float32)
        for jj, h in enumerate(heads):
            wqk[:, jj * 128 : jj * 128 + 64] = Wqkv[:, h * DH : (h + 1) * DH]
            wqk[:, jj * 128 + 64 : (jj + 1) * 128] = Wqkv[
                :, D + h * DH : D + (h + 1) * DH
            ]
            bqk[0:64, jj] = bqkv[h * DH : (h + 1) * DH] * 0.125
            bqk[64:128, jj] = bqkv[D + h * DH : D + (h + 1) * DH]
            wv[:, jj * DH : (jj + 1) * DH] = Wqkv[:, 2 * D + h * DH : 2 * D + (h + 1) * DH]
            bv[0, jj * DH : (jj + 1) * DH] = bqkv[2 * D + h * DH : 2 * D + (h + 1) * DH]
            woutp[jj * DH : (jj + 1) * DH, :] = Wout[h * DH : (h + 1) * DH, :]
            s = float(slopes_r[h])
            # s_r*idx premultiplied and split into a 12-significand-bit
            # value + residual (the f32r PE preserves x*1 only up to ~12
            # significand bits, so every aug row is (pre-rounded) * +-1)
            sk = np.float64(s) * q_idx.astype(np.float64)
            v = _round_mant(sk.astype(np.float32), 11)
            rv = _round_mant((sk - v.astype(np.float64)).astype(np.float32), 11)
            rq = v
            rr = rv
            augk[jj, 0, :] = _round_mant(mask_row, 11)
            augk[jj, 1, :] = v
            augk[jj, 2, :] = rv
            augk[jj, 3, :] = ones_row
            augk[jj, 4, :] = ones_row
            augqR[jj, 0, :] = ones_row
            augqR[jj, 1, :] = ones_row
            augqR[jj, 2, :] = ones_row
            augqR[jj, 3, :] = -rq
            augqR[jj, 4, :] = -rr
            augqL[jj, 0, :] = ones_row
            augqL[jj, 1, :] = -ones_row
            augqL[jj, 2, :] = -ones_row
            augqL[jj, 3, :] = rq
            augqL[jj, 4, :] = rr
            nslp[:, jj] = -s
        in_maps.append(
            {
                "xT": _round12(np.ascontiguousarray(x[b].T)),
                "wqk": wqk,
                "bqk": bqk,
                "wv": _round12(wv),
                "bv": _round12(bv),
                "woutp": woutp,
                "onesr": onesr,
                "augqR": augqR.copy(),
                "augqL": augqL.copy(),
                "augk": augk.copy(),
                "relb": relb,
                "nslp": nslp,
            }
        )
    return in_maps


def kernel(x, attn_mask, Wqkv, bqkv, Wout, bout):
    _ensure_concourse()
    from concourse.bass_utils import run_bass_kernel_spmd

    nc = _build_program()
    in_maps = _host_inputs(x, attn_mask, Wqkv, bqkv, Wout, bout)

    res = run_bass_kernel_spmd(
        nc,
        in_maps,
        list(range(N_CORES)),
        trace=bool(os.environ.get("BASS_TRACE")),
    )
    outs = [r["ypart"] for r in res.results]
    out = np.zeros((B, L, D), np.float32)
    for core in range(N_CORES):
        out[core // 4] += outs[core]
    out += np.asarray(bout, np.float32)[None, None, :]
    kernel.last_result = res
    if res.exec_time_ns is not None:
        kernel.last_exec_time_ns = res.exec_time_ns
    return out


# revision 30
# speedup vs baseline: 2.0896x; 1.0009x over previous
"""MultiHeadSelfAttention + ALiBi for Trainium2, SPMD over 8 NeuronCores.

Sharding: core c handles batch b = c // 4 and head group g = c % 4
(3 of the 12 heads, grouped so per-head ALiBi band sizes balance).
Each core computes y_partial[b] = ctx(heads_g) @ Wout[rows_g]; the host
sums the 4 partials per batch and adds bout.

All matmuls run in float32r (12-mantissa-bit fast mode, 4x the fp32
rate at free-dim >= 256).  f32r matmul operands must come from rounding
producers; DMA qualifies when the DRAM tensor is declared f32r, so the
host pre-rounds x/weights/aug rows to the 12-bit grid and the kernel
DMAs them straight into f32r tiles.  Device-computed operands (Q/K/V,
P=exp(S), ctx) round via their producing DVE/ACT instruction.

ALiBi handling under reduced precision: the per-head slope is
pre-rounded to 10 mantissa bits (s_r) and used consistently on both the
k and q aug rows, making slope rounding a per-head slope perturbation
(rel ~5e-4, harmless) instead of a softmax distortion.  The s_r*k_idx
aug row splits into v = round12(s_r*k) plus a residual row
r = s_r*k - v so large magnitudes survive the 12-bit grid exactly.

Device pipeline per core:
  1. QK^T = Wqk^T @ x^T  -> per head: Q'/8+bq into dual Q buffers, K+bk
     into K buffer.  V = x @ Wv + bv (ones column appended per head for
     softmax denominators).
  2. S^T blocks [128k x 512q]: matmul with augmented contraction rows
     carrying the attention mask bias and, off-diagonal, the ALiBi term
     -s_r*|q-k| (linear there).  Diagonal blocks get a fused DVE
     (rel_base * -s_r + S) pass.  exp() on ScalarE over 3-block groups,
     P^T @ V_aug accumulated in PSUM -> unnormalized ctx^T + denom row,
     copied to SBUF per (head, chunk).
  3. Batched normalization: one Ln over all 12 denom rows, then per-pair
     Exp(-x) (exp table loads once), K=1 broadcast matmul + in-place DVE
     multiply.  1/x = exp(-ln x) because the DVE reciprocal ops
     misbehave under this runtime.
  4. y = ctx^T.T @ Wout rows.  Blocks where ALiBi decays attention below
     ~2e-4 relative are skipped per the BANDS table (bout on the host).
"""

import math
import os

import numpy as np


def _ensure_concourse():
    try:
        import concourse  # noqa: F401
    except ImportError:
        import sys

        for p in ("/opt/trn_rl_repo", "/root/.axon_site/_ro/trn_rl_repo"):
            if os.path.isdir(p) and p not in sys.path:
                sys.path.insert(0, p)


B, L, D, H, DH = 2, 2048, 768, 12, 64
KT = L // 128  # 16 k-tiles
QC = L // 512  # 4 q-chunks
NH = 3  # heads per core
N_CORES = 8
GROUP_SIZE = 3  # exp/S group size in k-tiles (3 PSUM banks)

# Per head-slot key-tile bands per q-chunk (t_lo, t_hi_exclusive).  Slot
# 0 holds the wide-band heads (full attention); slots 1/2 hold heads
# whose ALiBi decays attention to ~exp(-36) beyond d_max = 36/slope
# keys.  The margin must cover worst-case Q.K swings (~±5 on each side
# of the exponent), so excluded blocks leak < ~2048*e^(-26) ~ 1e-8.
BANDS = [
    [(0, 16), (0, 16), (0, 16), (0, 16)],  # slot 0: full
    [(0, 9), (0, 13), (3, 16), (7, 16)],  # slot 1: d=576
    [(0, 6), (2, 10), (6, 14), (10, 16)],  # slot 2: d=204
]

# Head groups balanced by ALiBi band size (slopes below): each group gets
# one wide-band, one mid-band and one narrow-band head.
HEAD_GROUPS = [[4, 3, 0], [5, 2, 8], [6, 11, 9], [7, 1, 10]]

NEG_MASK = -1.0e9


def alibi_slopes(n_heads: int) -> np.ndarray:
    def slopes_pow2(n):
        start = 2 ** (-(2 ** -(math.log2(n) - 3)))
        return [start * start**i for i in range(n)]

    if math.log2(n_heads).is_integer():
        s = slopes_pow2(n_heads)
    else:
        cp = 2 ** int(math.floor(math.log2(n_heads)))
        s = slopes_pow2(cp) + slopes_pow2(2 * cp)[0::2][: n_heads - cp]
    return np.asarray(s, dtype=np.float32)


def _round_mant(x, bits):
    """Round fp32 values to `bits` explicit mantissa bits (RNE), i.e. onto
    the f32r grid (12) or safely within it (10)."""
    x = np.asarray(x, np.float32)
    b = x.view(np.uint32).copy()
    drop = 23 - bits
    b = b + (((b >> drop) & 1) + np.uint32((1 << (drop - 1)) - 1))
    b &= np.uint32(~((1 << drop) - 1) & 0xFFFFFFFF)
    return b.view(np.float32)


def _round10(x):
    return _round_mant(x, 10)


def _round12(x):
    return _round_mant(x, 12)


_PROGRAM_CACHE = {}


def _build_program():
    """Build the (shared, SPMD) Bass program once."""
    if "nc" in _PROGRAM_CACHE:
        return _PROGRAM_CACHE["nc"]

    _ensure_concourse()
    import concourse.mybir as mybir
    import concourse.tile as tile
    from concourse import bacc
    from concourse.bass import ts

    f32 = mybir.dt.float32
    f32r = mybir.dt.float32r
    bf16 = mybir.dt.bfloat16
    Exp = mybir.ActivationFunctionType.Exp
    Ln = mybir.ActivationFunctionType.Ln
    MULT = mybir.AluOpType.mult
    ADD = mybir.AluOpType.add

    nc = bacc.Bacc(None)

    # ---- DRAM I/O (f32r tensors arrive pre-rounded from the host) ----
    xT_d = nc.dram_tensor("xT", [D, L], f32r, kind="ExternalInput")
    wqk_d = nc.dram_tensor("wqk", [D, 2 * DH * NH], f32r, kind="ExternalInput")
    bqk_d = nc.dram_tensor("bqk", [128, NH], f32, kind="ExternalInput")
    wv_d = nc.dram_tensor("wv", [D, DH * NH], f32r, kind="ExternalInput")
    bv_d = nc.dram_tensor("bv", [1, DH * NH], f32r, kind="ExternalInput")
    woutp_d = nc.dram_tensor("woutp", [256, D], f32r, kind="ExternalInput")
    ones_d = nc.dram_tensor("onesr", [1, 128], f32r, kind="ExternalInput")
    augqR_d = nc.dram_tensor("augqR", [NH, 5, L], f32r, kind="ExternalInput")
    augqL_d = nc.dram_tensor("augqL", [NH, 5, L], f32r, kind="ExternalInput")
    augk_d = nc.dram_tensor("augk", [NH, 5, L], f32r, kind="ExternalInput")
    # rel_base[p, m, q'] = |q' - 128m - p| (unscaled; -s_r applied on DVE)
    relb_d = nc.dram_tensor("relb", [128, 4, 512], f32, kind="ExternalInput")
    # negslope[p, j] = -s_r of the core's head slot j (per-partition bcast)
    nslp_d = nc.dram_tensor("nslp", [128, NH], f32, kind="ExternalInput")
    y_d = nc.dram_tensor("ypart", [L, D], f32, kind="ExternalOutput")

    with tile.TileContext(nc) as tc:
        with tc.tile_pool(name="persist", bufs=1) as pp:
            # ---- persistent SBUF ----
            bqk_sb = pp.tile([128, NH], f32)
            nslp_sb = pp.tile([128, NH], f32)
            woutp_sb = pp.tile([128, 2, D], f32r)
            relb_sb = pp.tile([128, 4, 512], f32)
            ones_sb = pp.tile([1, 128], f32r)
            onesv_f = pp.tile([128, KT * NH], f32)
            V_sb = pp.tile([128, KT, NH, DH + 1], bf16)
            ctxA = pp.tile([128, L], f32r)  # heads 0,1 of group
            ctxB = pp.tile([64, L], f32r)  # head 2 of group
            # Per-head attention operand buffers.  The f32r PE rounds each
            # product to ~12 significand bits, so every aug product must be
            # a pre-rounded value times +-1: slope*idx terms are
            # premultiplied on the host (v/rv on the k side, rq/rr on the
            # q side, each split into a 12-sig-bit value plus residual).
            # Kbuf rows: 0-63 K^T, 64 mask, 65 v=rnd(s_r*k), 66 rv, 67 1, 68 1
            # QbufR rows: 0-63 Q'^T, 64 1, 65  1, 66  1, 67 -rq, 68 -rr
            # QbufL rows: 0-63 Q'^T, 64 1, 65 -1, 66 -1, 68  rq, 68  rr
            Kbuf = [
                pp.tile([69, L], f32r, tag=f"kb{j}", name=f"kb{j}") for j in range(NH)
            ]
            QbufR = [
                pp.tile([69, L], f32r, tag=f"qr{j}", name=f"qr{j}") for j in range(NH)
            ]
            QbufL = [
                pp.tile([69, L], f32r, tag=f"ql{j}", name=f"ql{j}") for j in range(NH)
            ]

            nc.sync.dma_start(bqk_sb[:], bqk_d[:])
            nc.sync.dma_start(nslp_sb[:], nslp_d[:])
            nc.sync.dma_start(woutp_sb[:], woutp_d.rearrange("(o p) n -> p o n", p=128))
            nc.sync.dma_start(relb_sb[:], relb_d[:])
            nc.sync.dma_start(ones_sb[:], ones_d[:])
            nc.gpsimd.memset(onesv_f[:], 1.0)
            # V softmax-denominator ones column (DVE copy rounds to f32r)
            nc.vector.tensor_copy(
                V_sb[:, :, :, DH : DH + 1].rearrange("p t h o -> p (t h o)"),
                onesv_f[:],
            )
            for j in range(NH):
                nc.sync.dma_start(QbufR[j][64:69, :], augqR_d[j])
                nc.sync.dma_start(QbufL[j][64:69, :], augqL_d[j])
                nc.sync.dma_start(Kbuf[j][64:69, :], augk_d[j])

            # Pre-touch DMA-loaded tiles consumed by TensorScalarPtr ops so
            # those ops carry a single sync wait (the walrus TS encoding
            # rejects multi-wait instructions).
            junk = pp.tile([1, 4], f32, name="junk")
            nc.vector.tensor_copy(junk[0:1, 0:1], bqk_sb[0:1, 0:1])
            nc.vector.tensor_copy(junk[0:1, 1:2], relb_sb[0:1, 0, 0:1])
            nc.vector.tensor_copy(junk[0:1, 2:3], nslp_sb[0:1, 0:1])

            # ---- stage 1: QKV projection ----
            with (
                tc.tile_pool(name="xpool", bufs=1) as xp,
                tc.tile_pool(name="ps1", bufs=4, space="PSUM") as ps1,
            ):
                wqk_sb = xp.tile([128, 6, 2 * DH * NH], f32r)
                wv_sb = xp.tile([128, 6, DH * NH], f32r)
                bv_sb = xp.tile([1, DH * NH], f32r)
                nc.sync.dma_start(wqk_sb[:], wqk_d.rearrange("(o p) m -> p o m", p=128))
                nc.sync.dma_start(wv_sb[:], wv_d.rearrange("(o p) m -> p o m", p=128))
                nc.sync.dma_start(bv_sb[:], bv_d[:])
                xT_sb = xp.tile([128, 6, L], f32r)
                for kt in range(6):
                    nc.sync.dma_start(
                        xT_sb[:, kt, :],
                        xT_d.rearrange("(o p) f -> p o f", p=128)[:, kt, :],
                    )

                # Q^T/K^T per head: PSUM [128, 512] = [Q^T_h; K^T_h] chunk
                for j in range(NH):
                    pcs = [
                        ps1.tile([128, 512], f32, tag="ps1", name=f"ps1c{c}")
                        for c in range(QC)
                    ]
                    for kt in range(6):
                        for c in range(QC):
                            nc.tensor.matmul(
                                pcs[c],
                                wqk_sb[:, kt, ts(j, 128)],
                                xT_sb[:, kt, ts(c, 512)],
                                start=(kt == 0),
                                stop=(kt == 5),
                            )
                    for c in range(QC):
                        ps = pcs[c]
                        cs = ts(c, 512)
                        nc.vector.tensor_scalar(
                            QbufR[j][0:64, cs],
                            ps[0:64, :],
                            0.125,
                            bqk_sb[0:64, j : j + 1],
                            MULT,
                            ADD,
                        )
                        nc.vector.tensor_copy(QbufL[j][0:64, cs], QbufR[j][0:64, cs])
                        nc.vector.tensor_scalar(
                            Kbuf[j][0:64, cs],
                            ps[64:128, :],
                            bqk_sb[64:128, j : j + 1],
                            None,
                            ADD,
                        )

                # V natural layout [l, d] + bias via K=1 matmul
                for lt in range(KT):
                    psv = ps1.tile([128, 512], f32, tag="ps1", name="psv")[:, : DH * NH]
                    for kt in range(6):
                        nc.tensor.matmul(
                            psv,
                            xT_sb[:, kt, ts(lt, 128)],
                            wv_sb[:, kt, :],
                            start=(kt == 0),
                            stop=False,
                        )
                    nc.tensor.matmul(
                        psv,
                        ones_sb[0:1, 0:128],
                        bv_sb[0:1, :],
                        start=False,
                        stop=True,
                    )
                    nc.scalar.copy(
                        V_sb[:, lt, :, 0:DH],
                        psv.rearrange("p (h x) -> p h x", x=DH),
                    )

            # ---- stage 2: attention ----
            # [1, 12*512] denom layout keeps every slice at partition base
            # 0 (activations reject input bases outside 0/32/64/96); the
            # pool wraps stages 2+2b and closes before stage 3.
            with tc.tile_pool(name="normp", bufs=1) as npool:
                denom_sb = npool.tile([1, NH * QC * 512], f32)
                lnr_sb = npool.tile([1, NH * QC * 512], f32)
                with (
                    tc.tile_pool(name="psS", bufs=2, space="PSUM") as psS,
                    tc.tile_pool(name="psO", bufs=2, space="PSUM") as psO,
                    tc.tile_pool(name="ptp", bufs=3) as ptp,
                ):
                  for j in range(NH):
                    for c in range(QC):
                        cs = ts(c, 512)
                        t_lo, t_hi = BANDS[j][c]
                        out_t = psO.tile([128, 512], f32, tag="outaug")
                        for t0 in range(t_lo, t_hi, GROUP_SIZE):
                            tn = min(GROUP_SIZE, t_hi - t0)
                            st = psS.tile([128, GROUP_SIZE * 512], f32, tag="st")
                            for i in range(tn):
                                t = t0 + i
                                js = ts(i, 512)
                                m = t - 4 * c
                                if 0 <= m < 4:  # diagonal block
                                    nc.tensor.matmul(
                                        st[:, js],
                                        Kbuf[j][0:65, ts(t, 128)],
                                        QbufR[j][0:65, cs],
                                        start=True,
                                        stop=True,
                                    )
                                    nc.vector.scalar_tensor_tensor(
                                        st[:, js],
                                        relb_sb[:, m, :],
                                        nslp_sb[:, j : j + 1],
                                        st[:, js],
                                        MULT,
                                        ADD,
                                    )
                                elif c > t // 4:  # keys before queries
                                    nc.tensor.matmul(
                                        st[:, js],
                                        Kbuf[j][0:69, ts(t, 128)],
                                        QbufR[j][0:69, cs],
                                        start=True,
                                        stop=True,
                                    )
                                else:  # keys after queries
                                    nc.tensor.matmul(
                                        st[:, js],
                                        Kbuf[j][0:69, ts(t, 128)],
                                        QbufL[j][0:69, cs],
                                        start=True,
                                        stop=True,
                                    )
                            pt = ptp.tile([128, GROUP_SIZE * 512], bf16, tag="pt")
                            nc.scalar.activation(
                                pt[:, : tn * 512], st[:, : tn * 512], Exp
                            )
                            for i in range(tn):
                                t = t0 + i
                                nc.tensor.matmul(
                                    out_t[0 : DH + 1, :],
                                    V_sb[:, t, j, :],
                                    pt[:, ts(i, 512)],
                                    start=(t == t_lo),
                                    stop=(t == t_hi - 1),
                                    skip_group_check=True,
                                )
                        # stash unnormalized ctx + denom; normalize later
                        if j < 2:
                            ctx_slice = ctxA[j * 64 : (j + 1) * 64, cs]
                        else:
                            ctx_slice = ctxB[0:64, cs]
                        nc.scalar.copy(ctx_slice, out_t[0:DH, :])
                        nc.scalar.copy(
                            denom_sb[0:1, ts(j * QC + c, 512)],
                            out_t[DH : DH + 1, :],
                        )

                # ---- stage 2b: batched normalization ----
                # 1/denom = exp(-ln(denom)); one Ln batch then per-pair
                # Exp (table loads once each), GpSimd partition
                # broadcast, in-place DVE mul.
                with tc.tile_pool(name="recp", bufs=2) as rp:
                    nc.scalar.activation(lnr_sb[:], denom_sb[:], Ln)
                    for j in range(NH):
                        for c in range(QC):
                            cs = ts(c, 512)
                            row = j * QC + c
                            # per-pair Exp lands at base partition 0 for
                            # the K=1 broadcast matmul (base 0/32/64)
                            rec = rp.tile([1, 512], f32, tag="rec")
                            nc.scalar.activation(
                                rec, lnr_sb[0:1, ts(row, 512)], Exp, scale=-1.0
                            )
                            recb = rp.tile([128, 512], f32, tag="recb")
                            nc.gpsimd.partition_broadcast(recb, rec)
                            # in-place mul needs equal SBUF base partitions
                            if j < 2:
                                ctx_slice = ctxA[j * 64 : (j + 1) * 64, cs]
                                recs = recb[j * 64 : (j + 1) * 64, :]
                            else:
                                ctx_slice = ctxB[0:64, cs]
                                recs = recb[0:64, :]
                            nc.vector.tensor_mul(ctx_slice, ctx_slice, recs)

            # ---- stage 3: output projection ----
            with (
                tc.tile_pool(name="ps3", bufs=2, space="PSUM") as ps3,
                tc.tile_pool(name="ysb", bufs=3) as yp,
            ):
                for lt in range(KT):
                    y = yp.tile([128, D], f32, tag="y")
                    for n0, nw in ((0, 512), (512, 256)):
                        ps = ps3.tile([128, 512], f32, tag="ps3", name="ps3t")[:, :nw]
                        nc.tensor.matmul(
                            ps,
                            ctxA[:, ts(lt, 128)],
                            woutp_sb[:, 0, n0 : n0 + nw],
                            start=True,
                            stop=False,
                        )
                        nc.tensor.matmul(
                            ps,
                            ctxB[0:64, ts(lt, 128)],
                            woutp_sb[0:64, 1, n0 : n0 + nw],
                            start=False,
                            stop=True,
                        )
                        nc.scalar.copy(y[:, n0 : n0 + nw], ps)
                    nc.sync.dma_start(y_d[ts(lt, 128), :], y)

    if not nc.is_finalized():
        nc.finalize()
    _PROGRAM_CACHE["nc"] = nc
    return nc


def _host_inputs(x, attn_mask, Wqkv, bqkv, Wout, bout):
    """Build the 8 per-core input dicts (f32r operands pre-rounded)."""
    slopes_r = _round10(alibi_slopes(H))
    x = np.asarray(x, dtype=np.float32)
    attn_mask = np.asarray(attn_mask)
    Wqkv = _round12(np.asarray(Wqkv, dtype=np.float32))
    bqkv = np.asarray(bqkv, dtype=np.float32)
    Wout = _round12(np.asarray(Wout, dtype=np.float32))
    bout = np.asarray(bout, dtype=np.float32)

    q_idx = np.arange(L, dtype=np.float32)
    ones_row = np.ones(L, dtype=np.float32)

    # rel_base[p, m, q'] = |q' - 128m - p|  (diagonal-block distance)
    p = np.arange(128, dtype=np.float32)[:, None, None]
    m = np.arange(4, dtype=np.float32)[None, :, None]
    qq = np.arange(512, dtype=np.float32)[None, None, :]
    relb = np.abs(qq - 128.0 * m - p).astype(np.float32)

    onesr = np.ones((1, 128), np.float32)

    in_maps = []
    for core in range(N_CORES):
        b = core // 4
        g = core % 4
        heads = HEAD_GROUPS[g]

        wqk = np.empty((D, 2 * DH * NH), np.float32)
        bqk = np.empty((128, NH), np.float32)
        wv = np.empty((D, DH * NH), np.float32)
        bv = np.empty((1, DH * NH), np.float32)
        woutp = np.zeros((256, D), np.float32)
        augk = np.empty((NH, 5, L), np.float32)
        augqR = np.empty((NH, 5, L), np.float32)
        augqL = np.empty((NH, 5, L), np.float32)
        nslp = np.empty((128, NH), np.float32)
        mask_row = np.where(attn_mask[b] == 0, NEG_MASK, 0.0).astype(np.float32)
        for jj, h in enumerate(heads):
            wqk[:, jj * 128 : jj * 128 + 64] = Wqkv[:, h * DH : (h + 1) * DH]
            wqk[:, jj * 128 + 64 : (jj + 1) * 128] = Wqkv[
                :, D + h * DH : D + (h + 1) * DH
            ]
            bqk[0:64, jj] = bqkv[h * DH : (h + 1) * DH] * 0.125
            bqk[64:128, jj] = bqkv[D + h * DH : D + (h + 1) * DH]
            wv[:, jj * DH : (jj + 1) * DH] = Wqkv[:, 2 * D + h * DH : 2 * D + (h + 1) * DH]
            bv[0, jj * DH : (jj + 1) * DH] = bqkv[2 * D + h * DH : 2 * D + (h + 1) * DH]
            woutp[jj * DH : (jj + 1) * DH, :] = Wout[h * DH : (h + 1) * DH, :]
            s = float(slopes_r[h])
            # s_r*idx premultiplied and split into a 12-significand-bit
            # value + residual (the f32r PE preserves x*1 only up to ~12
            # significand bits, so every aug row is (pre-rounded) * +-1)
            sk = np.float64(s) * q_idx.astype(np.float64)
            v = _round_mant(sk.astype(np.float32), 11)
            rv = _round_mant((sk - v.astype(np.float64)).astype(np.float32), 11)
            rq = v
            rr = rv
            augk[jj, 0, :] = _round_mant(mask_row, 11)
            augk[jj, 1, :] = v
            augk[jj, 2, :] = rv
            augk[jj, 3, :] = ones_row
            augk[jj, 4, :] = ones_row
            augqR[jj, 0, :] = ones_row
            augqR[jj, 1, :] = ones_row
            augqR[jj, 2, :] = ones_row
            augqR[jj, 3, :] = -rq
            augqR[jj, 4, :] = -rr
            augqL[jj, 0, :] = ones_row
            augqL[jj, 1, :] = -ones_row
            augqL[jj, 2, :] = -ones_row
            augqL[jj, 3, :] = rq
            augqL[jj, 4, :] = rr
            nslp[:, jj] = -s
        in_maps.append(
            {
                "xT": _round12(np.ascontiguousarray(x[b].T)),
                "wqk": wqk,
                "bqk": bqk,
                "wv": _round12(wv),
                "bv": _round12(bv),
                "woutp": woutp,
                "onesr": onesr,
                "augqR": augqR.copy(),
                "augqL": augqL.copy(),
                "augk": augk.copy(),
                "relb": relb,
                "nslp": nslp,
            }
        )
    return in_maps


def kernel(x, attn_mask, Wqkv, bqkv, Wout, bout):
    _ensure_concourse()
    from concourse.bass_utils import run_bass_kernel_spmd

    nc = _build_program()
    in_maps = _host_inputs(x, attn_mask, Wqkv, bqkv, Wout, bout)

    res = run_bass_kernel_spmd(
        nc,
        in_maps,
        list(range(N_CORES)),
        trace=bool(os.environ.get("BASS_TRACE")),
    )
    outs = [r["ypart"] for r in res.results]
    out = np.zeros((B, L, D), np.float32)
    for core in range(N_CORES):
        out[core // 4] += outs[core]
    out += np.asarray(bout, np.float32)[None, None, :]
    kernel.last_result = res
    if res.exec_time_ns is not None:
        kernel.last_exec_time_ns = res.exec_time_ns
    return out


# revision 32
# speedup vs baseline: 2.4650x; 1.1796x over previous
"""MultiHeadSelfAttention + ALiBi for Trainium2, SPMD over 8 NeuronCores.

Sharding: core c handles batch b = c // 4 and head group g = c % 4
(3 of the 12 heads, grouped so per-head ALiBi band sizes balance).
Each core computes y_partial[b] = ctx(heads_g) @ Wout[rows_g]; the host
sums the 4 partials per batch and adds bout.

All matmuls run in float32r (12-mantissa-bit fast mode, 4x the fp32
rate at free-dim >= 256).  f32r matmul operands must come from rounding
producers; DMA qualifies when the DRAM tensor is declared f32r, so the
host pre-rounds x/weights/aug rows to the 12-bit grid and the kernel
DMAs them straight into f32r tiles.  Device-computed operands (Q/K/V,
P=exp(S), ctx) round via their producing DVE/ACT instruction.

ALiBi handling under reduced precision: the per-head slope is
pre-rounded to 10 mantissa bits (s_r) and used consistently on both the
k and q aug rows, making slope rounding a per-head slope perturbation
(rel ~5e-4, harmless) instead of a softmax distortion.  The s_r*k_idx
aug row splits into v = round12(s_r*k) plus a residual row
r = s_r*k - v so large magnitudes survive the 12-bit grid exactly.

Device pipeline per core:
  1. QK^T = Wqk^T @ x^T  -> per head: Q'/8+bq into dual Q buffers, K+bk
     into K buffer.  V = x @ Wv + bv (ones column appended per head for
     softmax denominators).
  2. S^T blocks [128k x 512q]: matmul with augmented contraction rows
     carrying the attention mask bias and, off-diagonal, the ALiBi term
     -s_r*|q-k| (linear there).  Diagonal blocks get a fused DVE
     (rel_base * -s_r + S) pass.  exp() on ScalarE over 3-block groups,
     P^T @ V_aug accumulated in PSUM -> unnormalized ctx^T + denom row,
     copied to SBUF per (head, chunk).
  3. Batched normalization: one Ln over all 12 denom rows, then per-pair
     Exp(-x) (exp table loads once), K=1 broadcast matmul + in-place DVE
     multiply.  1/x = exp(-ln x) because the DVE reciprocal ops
     misbehave under this runtime.
  4. y = ctx^T.T @ Wout rows.  Blocks where ALiBi decays attention below
     ~2e-4 relative are skipped per the BANDS table (bout on the host).
"""

import math
import os

import numpy as np


def _ensure_concourse():
    try:
        import concourse  # noqa: F401
    except ImportError:
        import sys

        for p in ("/opt/trn_rl_repo", "/root/.axon_site/_ro/trn_rl_repo"):
            if os.path.isdir(p) and p not in sys.path:
                sys.path.insert(0, p)


B, L, D, H, DH = 2, 2048, 768, 12, 64
KT = L // 128  # 16 k-tiles
QC = L // 512  # 4 q-chunks
NH = 3  # heads per core
N_CORES = 8
GROUP_SIZE = 2  # exp/S group size in k-tiles (2 PSUM banks)

# Per head-slot key-tile bands per q-chunk (t_lo, t_hi_exclusive).  Slot
# 0 holds the wide-band heads (full attention); slots 1/2 hold heads
# whose ALiBi decays attention to ~exp(-36) beyond d_max = 36/slope
# keys.  The margin must cover worst-case Q.K swings (~±5 on each side
# of the exponent), so excluded blocks leak < ~2048*e^(-26) ~ 1e-8.
BANDS = [
    [(0, 16), (0, 16), (0, 16), (0, 16)],  # slot 0: full
    [(0, 9), (0, 13), (3, 16), (7, 16)],  # slot 1: d=576
    [(0, 6), (2, 10), (6, 14), (10, 16)],  # slot 2: d=204
]

# Head groups balanced by ALiBi band size (slopes below): each group gets
# one wide-band, one mid-band and one narrow-band head.
HEAD_GROUPS = [[4, 3, 0], [5, 2, 8], [6, 11, 9], [7, 1, 10]]

NEG_MASK = -1.0e9


def alibi_slopes(n_heads: int) -> np.ndarray:
    def slopes_pow2(n):
        start = 2 ** (-(2 ** -(math.log2(n) - 3)))
        return [start * start**i for i in range(n)]

    if math.log2(n_heads).is_integer():
        s = slopes_pow2(n_heads)
    else:
        cp = 2 ** int(math.floor(math.log2(n_heads)))
        s = slopes_pow2(cp) + slopes_pow2(2 * cp)[0::2][: n_heads - cp]
    return np.asarray(s, dtype=np.float32)


def _round_mant(x, bits):
    """Round fp32 values to `bits` explicit mantissa bits (RNE), i.e. onto
    the f32r grid (12) or safely within it (10)."""
    x = np.asarray(x, np.float32)
    b = x.view(np.uint32).copy()
    drop = 23 - bits
    b = b + (((b >> drop) & 1) + np.uint32((1 << (drop - 1)) - 1))
    b &= np.uint32(~((1 << drop) - 1) & 0xFFFFFFFF)
    return b.view(np.float32)


def _round10(x):
    return _round_mant(x, 10)


def _round12(x):
    return _round_mant(x, 12)


_PROGRAM_CACHE = {}


def _build_program():
    """Build the (shared, SPMD) Bass program once."""
    if "nc" in _PROGRAM_CACHE:
        return _PROGRAM_CACHE["nc"]

    _ensure_concourse()
    import concourse.mybir as mybir
    import concourse.tile as tile
    from concourse import bacc
    from concourse.bass import ts

    f32 = mybir.dt.float32
    f32r = mybir.dt.float32r
    bf16 = mybir.dt.bfloat16
    Exp = mybir.ActivationFunctionType.Exp
    Ln = mybir.ActivationFunctionType.Ln
    MULT = mybir.AluOpType.mult
    ADD = mybir.AluOpType.add

    nc = bacc.Bacc(None)

    # ---- DRAM I/O (f32r tensors arrive pre-rounded from the host) ----
    xT_d = nc.dram_tensor("xT", [D, L], f32r, kind="ExternalInput")
    wqk_d = nc.dram_tensor("wqk", [D, 2 * DH * NH], f32r, kind="ExternalInput")
    bqk_d = nc.dram_tensor("bqk", [128, NH], f32, kind="ExternalInput")
    wv_d = nc.dram_tensor("wv", [D, DH * NH], f32r, kind="ExternalInput")
    bv_d = nc.dram_tensor("bv", [1, DH * NH], f32r, kind="ExternalInput")
    woutp_d = nc.dram_tensor("woutp", [256, D], f32r, kind="ExternalInput")
    ones_d = nc.dram_tensor("onesr", [1, 128], f32r, kind="ExternalInput")
    augqR_d = nc.dram_tensor("augqR", [NH, 5, L], f32r, kind="ExternalInput")
    augqL_d = nc.dram_tensor("augqL", [NH, 5, L], f32r, kind="ExternalInput")
    augk_d = nc.dram_tensor("augk", [NH, 5, L], f32r, kind="ExternalInput")
    # rel_base[p, m, q'] = |q' - 128m - p| (unscaled; -s_r applied on DVE)
    relb_d = nc.dram_tensor("relb", [128, 4, 512], f32, kind="ExternalInput")
    # negslope[p, j] = -s_r of the core's head slot j (per-partition bcast)
    nslp_d = nc.dram_tensor("nslp", [128, NH], f32, kind="ExternalInput")
    y_d = nc.dram_tensor("ypart", [L, D], f32, kind="ExternalOutput")

    with tile.TileContext(nc) as tc:
        with tc.tile_pool(name="persist", bufs=1) as pp:
            # ---- persistent SBUF ----
            bqk_sb = pp.tile([128, NH], f32)
            nslp_sb = pp.tile([128, NH], f32)
            woutp_sb = pp.tile([128, 2, D], f32r)
            relb_sb = pp.tile([128, 4, 512], f32)
            ones_sb = pp.tile([1, 128], f32r)
            onesv_f = pp.tile([128, KT * NH], f32)
            V_sb = pp.tile([128, KT, NH, DH + 1], bf16)
            ctxA = pp.tile([128, L], f32r)  # heads 0,1 of group
            ctxB = pp.tile([64, L], f32r)  # head 2 of group
            # Per-head attention operand buffers.  The f32r PE rounds each
            # product to ~12 significand bits, so every aug product must be
            # a pre-rounded value times +-1: slope*idx terms are
            # premultiplied on the host (v/rv on the k side, rq/rr on the
            # q side, each split into a 12-sig-bit value plus residual).
            # Kbuf rows: 0-63 K^T, 64 mask, 65 v=rnd(s_r*k), 66 rv, 67 1, 68 1
            # QbufR rows: 0-63 Q'^T, 64 1, 65  1, 66  1, 67 -rq, 68 -rr
            # QbufL rows: 0-63 Q'^T, 64 1, 65 -1, 66 -1, 68  rq, 68  rr
            Kbuf = [
                pp.tile([69, L], f32r, tag=f"kb{j}", name=f"kb{j}") for j in range(NH)
            ]
            QbufR = [
                pp.tile([69, L], f32r, tag=f"qr{j}", name=f"qr{j}") for j in range(NH)
            ]
            QbufL = [
                pp.tile([69, L], f32r, tag=f"ql{j}", name=f"ql{j}") for j in range(NH)
            ]

            nc.sync.dma_start(bqk_sb[:], bqk_d[:])
            nc.sync.dma_start(nslp_sb[:], nslp_d[:])
            nc.sync.dma_start(woutp_sb[:], woutp_d.rearrange("(o p) n -> p o n", p=128))
            nc.sync.dma_start(relb_sb[:], relb_d[:])
            nc.sync.dma_start(ones_sb[:], ones_d[:])
            nc.gpsimd.memset(onesv_f[:], 1.0)
            # V softmax-denominator ones column (DVE copy rounds to f32r)
            nc.vector.tensor_copy(
                V_sb[:, :, :, DH : DH + 1].rearrange("p t h o -> p (t h o)"),
                onesv_f[:],
            )
            for j in range(NH):
                nc.sync.dma_start(QbufR[j][64:69, :], augqR_d[j])
                nc.sync.dma_start(QbufL[j][64:69, :], augqL_d[j])
                nc.sync.dma_start(Kbuf[j][64:69, :], augk_d[j])

            # Pre-touch DMA-loaded tiles consumed by TensorScalarPtr ops so
            # those ops carry a single sync wait (the walrus TS encoding
            # rejects multi-wait instructions).
            junk = pp.tile([1, 4], f32, name="junk")
            nc.vector.tensor_copy(junk[0:1, 0:1], bqk_sb[0:1, 0:1])
            nc.vector.tensor_copy(junk[0:1, 1:2], relb_sb[0:1, 0, 0:1])
            nc.vector.tensor_copy(junk[0:1, 2:3], nslp_sb[0:1, 0:1])

            # ---- stage 1: QKV projection ----
            with (
                tc.tile_pool(name="xpool", bufs=1) as xp,
                tc.tile_pool(name="ps1", bufs=4, space="PSUM") as ps1,
            ):
                wqk_sb = xp.tile([128, 6, 2 * DH * NH], f32r)
                wv_sb = xp.tile([128, 6, DH * NH], f32r)
                bv_sb = xp.tile([1, DH * NH], f32r)
                nc.sync.dma_start(wqk_sb[:], wqk_d.rearrange("(o p) m -> p o m", p=128))
                nc.sync.dma_start(wv_sb[:], wv_d.rearrange("(o p) m -> p o m", p=128))
                nc.sync.dma_start(bv_sb[:], bv_d[:])
                xT_sb = xp.tile([128, 6, L], f32r)
                for kt in range(6):
                    nc.sync.dma_start(
                        xT_sb[:, kt, :],
                        xT_d.rearrange("(o p) f -> p o f", p=128)[:, kt, :],
                    )

                # Q^T/K^T per head: PSUM [128, 512] = [Q^T_h; K^T_h] chunk
                for j in range(NH):
                    pcs = [
                        ps1.tile([128, 512], f32, tag="ps1", name=f"ps1c{c}")
                        for c in range(QC)
                    ]
                    for kt in range(6):
                        for c in range(QC):
                            nc.tensor.matmul(
                                pcs[c],
                                wqk_sb[:, kt, ts(j, 128)],
                                xT_sb[:, kt, ts(c, 512)],
                                start=(kt == 0),
                                stop=(kt == 5),
                            )
                    for c in range(QC):
                        ps = pcs[c]
                        cs = ts(c, 512)
                        nc.vector.tensor_scalar(
                            QbufR[j][0:64, cs],
                            ps[0:64, :],
                            0.125,
                            bqk_sb[0:64, j : j + 1],
                            MULT,
                            ADD,
                        )
                        nc.vector.tensor_copy(QbufL[j][0:64, cs], QbufR[j][0:64, cs])
                        nc.vector.tensor_scalar(
                            Kbuf[j][0:64, cs],
                            ps[64:128, :],
                            bqk_sb[64:128, j : j + 1],
                            None,
                            ADD,
                        )

                # V natural layout [l, d] + bias via K=1 matmul
                for lt in range(KT):
                    psv = ps1.tile([128, 512], f32, tag="ps1", name="psv")[:, : DH * NH]
                    for kt in range(6):
                        nc.tensor.matmul(
                            psv,
                            xT_sb[:, kt, ts(lt, 128)],
                            wv_sb[:, kt, :],
                            start=(kt == 0),
                            stop=False,
                        )
                    nc.tensor.matmul(
                        psv,
                        ones_sb[0:1, 0:128],
                        bv_sb[0:1, :],
                        start=False,
                        stop=True,
                    )
                    nc.scalar.copy(
                        V_sb[:, lt, :, 0:DH],
                        psv.rearrange("p (h x) -> p h x", x=DH),
                    )

            # ---- stage 2: attention ----
            # [1, 12*512] denom layout keeps every slice at partition base
            # 0 (activations reject input bases outside 0/32/64/96); the
            # pool wraps stages 2+2b and closes before stage 3.
            with tc.tile_pool(name="normp", bufs=1) as npool:
                denom_sb = npool.tile([1, NH * QC * 512], f32)
                lnr_sb = npool.tile([1, NH * QC * 512], f32)
                with (
                    tc.tile_pool(name="psS", bufs=3, space="PSUM") as psS,
                    tc.tile_pool(name="psO", bufs=2, space="PSUM") as psO,
                    tc.tile_pool(name="ptp", bufs=3) as ptp,
                ):

                    def emit_sgroup(j, c, t0, tn, st):
                        cs = ts(c, 512)
                        for i in range(tn):
                            t = t0 + i
                            js = ts(i, 512)
                            m = t - 4 * c
                            if 0 <= m < 4:  # diagonal block
                                nc.tensor.matmul(
                                    st[:, js],
                                    Kbuf[j][0:65, ts(t, 128)],
                                    QbufR[j][0:65, cs],
                                    start=True,
                                    stop=True,
                                )
                                nc.vector.scalar_tensor_tensor(
                                    st[:, js],
                                    relb_sb[:, m, :],
                                    nslp_sb[:, j : j + 1],
                                    st[:, js],
                                    MULT,
                                    ADD,
                                )
                            elif c > t // 4:  # keys before queries
                                nc.tensor.matmul(
                                    st[:, js],
                                    Kbuf[j][0:69, ts(t, 128)],
                                    QbufR[j][0:69, cs],
                                    start=True,
                                    stop=True,
                                )
                            else:  # keys after queries
                                nc.tensor.matmul(
                                    st[:, js],
                                    Kbuf[j][0:69, ts(t, 128)],
                                    QbufL[j][0:69, cs],
                                    start=True,
                                    stop=True,
                                )

                    def emit_exp_pv(j, c, t0, tn, st, out_t, t_lo, t_hi):
                        pt = ptp.tile(
                            [128, GROUP_SIZE * 512], bf16, tag="pt", name="pt"
                        )
                        nc.scalar.activation(pt[:, : tn * 512], st[:, : tn * 512], Exp)
                        for i in range(tn):
                            t = t0 + i
                            nc.tensor.matmul(
                                out_t[0 : DH + 1, :],
                                V_sb[:, t, j, :],
                                pt[:, ts(i, 512)],
                                start=(t == t_lo),
                                stop=(t == t_hi - 1),
                                skip_group_check=True,
                            )

                    def groups_of(j, c):
                        t_lo, t_hi = BANDS[j][c]
                        return [
                            (t0, min(GROUP_SIZE, t_hi - t0))
                            for t0 in range(t_lo, t_hi, GROUP_SIZE)
                        ]

                    # Software-pipeline each (head, chunk) pair: emit the
                    # S-matmuls one group ahead of exp/PV so the PE always
                    # has queued work while exp runs (a continuously-fed PE
                    # opens the HAM clock gate: 2.4GHz vs 1.2GHz).  Only
                    # one PSUM accumulation group (out_t) is open at a time
                    # (two open groups wedge the exec unit).
                    for j in range(NH):
                        for c in range(QC):
                            cs = ts(c, 512)
                            t_lo, t_hi = BANDS[j][c]
                            out_t = psO.tile(
                                [128, 512], f32, tag="outaug", name="outaug"
                            )
                            gl = groups_of(j, c)
                            sts = []
                            for g, (t0, tn) in enumerate(gl):
                                st = psS.tile(
                                    [128, GROUP_SIZE * 512], f32,
                                    tag="st", name="st",
                                )
                                sts.append(st)
                                emit_sgroup(j, c, t0, tn, st)
                                if g >= 1:
                                    emit_exp_pv(
                                        j, c, *gl[g - 1], sts[g - 1], out_t,
                                        t_lo, t_hi,
                                    )
                            emit_exp_pv(
                                j, c, *gl[-1], sts[-1], out_t, t_lo, t_hi
                            )
                            # stash unnormalized ctx + denom; normalize later
                            if j < 2:
                                ctx_slice = ctxA[j * 64 : (j + 1) * 64, cs]
                            else:
                                ctx_slice = ctxB[0:64, cs]
                            nc.scalar.copy(ctx_slice, out_t[0:DH, :])
                            nc.scalar.copy(
                                denom_sb[0:1, ts(j * QC + c, 512)],
                                out_t[DH : DH + 1, :],
                            )

                # ---- stage 2b: batched normalization ----
                # 1/denom = exp(-ln(denom)); one Ln batch then per-pair
                # Exp (table loads once each), GpSimd partition
                # broadcast, in-place DVE mul.
                with tc.tile_pool(name="recp", bufs=2) as rp:
                    nc.scalar.activation(lnr_sb[:], denom_sb[:], Ln)
                    for j in range(NH):
                        for c in range(QC):
                            cs = ts(c, 512)
                            row = j * QC + c
                            # per-pair Exp lands at base partition 0 for
                            # the K=1 broadcast matmul (base 0/32/64)
                            rec = rp.tile([1, 512], f32, tag="rec")
                            nc.scalar.activation(
                                rec, lnr_sb[0:1, ts(row, 512)], Exp, scale=-1.0
                            )
                            recb = rp.tile([128, 512], f32, tag="recb")
                            nc.gpsimd.partition_broadcast(recb, rec)
                            # in-place mul needs equal SBUF base partitions
                            if j < 2:
                                ctx_slice = ctxA[j * 64 : (j + 1) * 64, cs]
                                recs = recb[j * 64 : (j + 1) * 64, :]
                            else:
                                ctx_slice = ctxB[0:64, cs]
                                recs = recb[0:64, :]
                            nc.vector.tensor_mul(ctx_slice, ctx_slice, recs)

            # ---- stage 3: output projection ----
            with (
                tc.tile_pool(name="ps3", bufs=2, space="PSUM") as ps3,
                tc.tile_pool(name="ysb", bufs=3) as yp,
            ):
                for lt in range(KT):
                    y = yp.tile([128, D], f32, tag="y")
                    for n0, nw in ((0, 512), (512, 256)):
                        ps = ps3.tile([128, 512], f32, tag="ps3", name="ps3t")[:, :nw]
                        nc.tensor.matmul(
                            ps,
                            ctxA[:, ts(lt, 128)],
                            woutp_sb[:, 0, n0 : n0 + nw],
                            start=True,
                            stop=False,
                        )
                        nc.tensor.matmul(
                            ps,
                            ctxB[0:64, ts(lt, 128)],
                            woutp_sb[0:64, 1, n0 : n0 + nw],
                            start=False,
                            stop=True,
                        )
                        nc.scalar.copy(y[:, n0 : n0 + nw], ps)
                    nc.sync.dma_start(y_d[ts(lt, 128), :], y)

    if not nc.is_finalized():
        nc.finalize()
    _PROGRAM_CACHE["nc"] = nc
    return nc


def _host_inputs(x, attn_mask, Wqkv, bqkv, Wout, bout):
    """Build the 8 per-core input dicts (f32r operands pre-rounded)."""
    slopes_r = _round10(alibi_slopes(H))
    x = np.asarray(x, dtype=np.float32)
    attn_mask = np.asarray(attn_mask)
    Wqkv = _round12(np.asarray(Wqkv, dtype=np.float32))
    bqkv = np.asarray(bqkv, dtype=np.float32)
    Wout = _round12(np.asarray(Wout, dtype=np.float32))
    bout = np.asarray(bout, dtype=np.float32)

    q_idx = np.arange(L, dtype=np.float32)
    ones_row = np.ones(L, dtype=np.float32)

    # rel_base[p, m, q'] = |q' - 128m - p|  (diagonal-block distance)
    p = np.arange(128, dtype=np.float32)[:, None, None]
    m = np.arange(4, dtype=np.float32)[None, :, None]
    qq = np.arange(512, dtype=np.float32)[None, None, :]
    relb = np.abs(qq - 128.0 * m - p).astype(np.float32)

    onesr = np.ones((1, 128), np.float32)

    in_maps = []
    for core in range(N_CORES):
        b = core // 4
        g = core % 4
        heads = HEAD_GROUPS[g]

        wqk = np.empty((D, 2 * DH * NH), np.float32)
        bqk = np.empty((128, NH), np.float32)
        wv = np.empty((D, DH * NH), np.float32)
        bv = np.empty((1, DH * NH), np.float32)
        woutp = np.zeros((256, D), np.float32)
        augk = np.empty((NH, 5, L), np.float32)
        augqR = np.empty((NH, 5, L), np.float32)
        augqL = np.empty((NH, 5, L), np.float32)
        nslp = np.empty((128, NH), np.float32)
        mask_row = np.where(attn_mask[b] == 0, NEG_MASK, 0.0).astype(np.float32)
        for jj, h in enumerate(heads):
            wqk[:, jj * 128 : jj * 128 + 64] = Wqkv[:, h * DH : (h + 1) * DH]
            wqk[:, jj * 128 + 64 : (jj + 1) * 128] = Wqkv[
                :, D + h * DH : D + (h + 1) * DH
            ]
            bqk[0:64, jj] = bqkv[h * DH : (h + 1) * DH] * 0.125
            bqk[64:128, jj] = bqkv[D + h * DH : D + (h + 1) * DH]
            wv[:, jj * DH : (jj + 1) * DH] = Wqkv[:, 2 * D + h * DH : 2 * D + (h + 1) * DH]
            bv[0, jj * DH : (jj + 1) * DH] = bqkv[2 * D + h * DH : 2 * D + (h + 1) * DH]
            woutp[jj * DH : (jj + 1) * DH, :] = Wout[h * DH : (h + 1) * DH, :]
            s = float(slopes_r[h])
            # s_r*idx premultiplied and split into a 12-significand-bit
            # value + residual (the f32r PE preserves x*1 only up to ~12
            # significand bits, so every aug row is (pre-rounded) * +-1)
            sk = np.float64(s) * q_idx.astype(np.float64)
            v = _round_mant(sk.astype(np.float32), 11)
            rv = _round_mant((sk - v.astype(np.float64)).astype(np.float32), 11)
            rq = v
            rr = rv
            augk[jj, 0, :] = _round_mant(mask_row, 11)
            augk[jj, 1, :] = v
            augk[jj, 2, :] = rv
            augk[jj, 3, :] = ones_row
            augk[jj, 4, :] = ones_row
            augqR[jj, 0, :] = ones_row
            augqR[jj, 1, :] = ones_row
            augqR[jj, 2, :] = ones_row
            augqR[jj, 3, :] = -rq
            augqR[jj, 4, :] = -rr
            augqL[jj, 0, :] = ones_row
            augqL[jj, 1, :] = -ones_row
            augqL[jj, 2, :] = -ones_row
            augqL[jj, 3, :] = rq
            augqL[jj, 4, :] = rr
            nslp[:, jj] = -s
        in_maps.append(
            {
                "xT": _round12(np.ascontiguousarray(x[b].T)),
                "wqk": wqk,
                "bqk": bqk,
                "wv": _round12(wv),
                "bv": _round12(bv),
                "woutp": woutp,
                "onesr": onesr,
                "augqR": augqR.copy(),
                "augqL": augqL.copy(),
                "augk": augk.copy(),
                "relb": relb,
                "nslp": nslp,
            }
        )
    return in_maps


def kernel(x, attn_mask, Wqkv, bqkv, Wout, bout):
    _ensure_concourse()
    from concourse.bass_utils import run_bass_kernel_spmd

    nc = _build_program()
    in_maps = _host_inputs(x, attn_mask, Wqkv, bqkv, Wout, bout)

    res = run_bass_kernel_spmd(
        nc,
        in_maps,
        list(range(N_CORES)),
        trace=bool(os.environ.get("BASS_TRACE")),
    )
    outs = [r["ypart"] for r in res.results]
    out = np.zeros((B, L, D), np.float32)
    for core in range(N_CORES):
        out[core // 4] += outs[core]
    out += np.asarray(bout, np.float32)[None, None, :]
    kernel.last_result = res
    if res.exec_time_ns is not None:
        kernel.last_exec_time_ns = res.exec_time_ns
    return out


# revision 33
# speedup vs baseline: 2.4661x; 1.0005x over previous
"""MultiHeadSelfAttention + ALiBi for Trainium2, SPMD over 8 NeuronCores.

Sharding: core c handles batch b = c // 4 and head group g = c % 4
(3 of the 12 heads, grouped so per-head ALiBi band sizes balance).
Each core computes y_partial[b] = ctx(heads_g) @ Wout[rows_g]; the host
sums the 4 partials per batch and adds bout.

All matmuls run in float32r (12-mantissa-bit fast mode, 4x the fp32
rate at free-dim >= 256).  f32r matmul operands must come from rounding
producers; DMA qualifies when the DRAM tensor is declared f32r, so the
host pre-rounds x/weights/aug rows to the 12-bit grid and the kernel
DMAs them straight into f32r tiles.  Device-computed operands (Q/K/V,
P=exp(S), ctx) round via their producing DVE/ACT instruction.

ALiBi handling under reduced precision: the per-head slope is
pre-rounded to 10 mantissa bits (s_r) and used consistently on both the
k and q aug rows, making slope rounding a per-head slope perturbation
(rel ~5e-4, harmless) instead of a softmax distortion.  The s_r*k_idx
aug row splits into v = round12(s_r*k) plus a residual row
r = s_r*k - v so large magnitudes survive the 12-bit grid exactly.

Device pipeline per core:
  1. QK^T = Wqk^T @ x^T  -> per head: Q'/8+bq into dual Q buffers, K+bk
     into K buffer.  V = x @ Wv + bv (ones column appended per head for
     softmax denominators).
  2. S^T blocks [128k x 512q]: matmul with augmented contraction rows
     carrying the attention mask bias and, off-diagonal, the ALiBi term
     -s_r*|q-k| (linear there).  Diagonal blocks get a fused DVE
     (rel_base * -s_r + S) pass.  exp() on ScalarE over 3-block groups,
     P^T @ V_aug accumulated in PSUM -> unnormalized ctx^T + denom row,
     copied to SBUF per (head, chunk).
  3. Batched normalization: one Ln over all 12 denom rows, then per-pair
     Exp(-x) (exp table loads once), K=1 broadcast matmul + in-place DVE
     multiply.  1/x = exp(-ln x) because the DVE reciprocal ops
     misbehave under this runtime.
  4. y = ctx^T.T @ Wout rows.  Blocks where ALiBi decays attention below
     ~2e-4 relative are skipped per the BANDS table (bout on the host).
"""

import math
import os

import numpy as np


def _ensure_concourse():
    try:
        import concourse  # noqa: F401
    except ImportError:
        import sys

        for p in ("/opt/trn_rl_repo", "/root/.axon_site/_ro/trn_rl_repo"):
            if os.path.isdir(p) and p not in sys.path:
                sys.path.insert(0, p)


B, L, D, H, DH = 2, 2048, 768, 12, 64
KT = L // 128  # 16 k-tiles
QC = L // 512  # 4 q-chunks
NH = 3  # heads per core
N_CORES = 8
GROUP_SIZE = 2  # exp/S group size in k-tiles (2 PSUM banks)

# Per head-slot key-tile bands per q-chunk (t_lo, t_hi_exclusive).  Slot
# 0 holds the wide-band heads (full attention); slots 1/2 hold heads
# whose ALiBi decays attention to ~exp(-36) beyond d_max = 36/slope
# keys.  The margin must cover worst-case Q.K swings (~±5 on each side
# of the exponent), so excluded blocks leak < ~2048*e^(-26) ~ 1e-8.
BANDS = [
    [(0, 16), (0, 16), (0, 16), (0, 16)],  # slot 0: full
    [(0, 9), (0, 13), (3, 16), (7, 16)],  # slot 1: d=576
    [(0, 6), (2, 10), (6, 14), (10, 16)],  # slot 2: d=204
]

# Head groups balanced by ALiBi band size (slopes below): each group gets
# one wide-band, one mid-band and one narrow-band head.
HEAD_GROUPS = [[4, 3, 0], [5, 2, 8], [6, 11, 9], [7, 1, 10]]

NEG_MASK = -1.0e9


def alibi_slopes(n_heads: int) -> np.ndarray:
    def slopes_pow2(n):
        start = 2 ** (-(2 ** -(math.log2(n) - 3)))
        return [start * start**i for i in range(n)]

    if math.log2(n_heads).is_integer():
        s = slopes_pow2(n_heads)
    else:
        cp = 2 ** int(math.floor(math.log2(n_heads)))
        s = slopes_pow2(cp) + slopes_pow2(2 * cp)[0::2][: n_heads - cp]
    return np.asarray(s, dtype=np.float32)


def _round_mant(x, bits):
    """Round fp32 values to `bits` explicit mantissa bits (RNE), i.e. onto
    the f32r grid (12) or safely within it (10)."""
    x = np.asarray(x, np.float32)
    b = x.view(np.uint32).copy()
    drop = 23 - bits
    b = b + (((b >> drop) & 1) + np.uint32((1 << (drop - 1)) - 1))
    b &= np.uint32(~((1 << drop) - 1) & 0xFFFFFFFF)
    return b.view(np.float32)


def _round10(x):
    return _round_mant(x, 10)


def _round12(x):
    return _round_mant(x, 12)


_PROGRAM_CACHE = {}


def _build_program():
    """Build the (shared, SPMD) Bass program once."""
    if "nc" in _PROGRAM_CACHE:
        return _PROGRAM_CACHE["nc"]

    _ensure_concourse()
    import concourse.mybir as mybir
    import concourse.tile as tile
    from concourse import bacc
    from concourse.bass import ts

    f32 = mybir.dt.float32
    f32r = mybir.dt.float32r
    bf16 = mybir.dt.bfloat16
    Exp = mybir.ActivationFunctionType.Exp
    Ln = mybir.ActivationFunctionType.Ln
    MULT = mybir.AluOpType.mult
    ADD = mybir.AluOpType.add

    nc = bacc.Bacc(None)

    # ---- DRAM I/O (f32r tensors arrive pre-rounded from the host) ----
    xT_d = nc.dram_tensor("xT", [D, L], f32r, kind="ExternalInput")
    wqk_d = nc.dram_tensor("wqk", [D, 2 * DH * NH], f32r, kind="ExternalInput")
    bqk_d = nc.dram_tensor("bqk", [128, NH], f32, kind="ExternalInput")
    wv_d = nc.dram_tensor("wv", [D, DH * NH], f32r, kind="ExternalInput")
    bv_d = nc.dram_tensor("bv", [1, DH * NH], f32r, kind="ExternalInput")
    woutp_d = nc.dram_tensor("woutp", [256, D], f32r, kind="ExternalInput")
    ones_d = nc.dram_tensor("onesr", [1, 128], f32r, kind="ExternalInput")
    augqR_d = nc.dram_tensor("augqR", [NH, 5, L], f32r, kind="ExternalInput")
    augqL_d = nc.dram_tensor("augqL", [NH, 5, L], f32r, kind="ExternalInput")
    augk_d = nc.dram_tensor("augk", [NH, 5, L], f32r, kind="ExternalInput")
    # rel_base[p, m, q'] = |q' - 128m - p| (unscaled; -s_r applied on DVE)
    relb_d = nc.dram_tensor("relb", [128, 4, 512], f32, kind="ExternalInput")
    # negslope[p, j] = -s_r of the core's head slot j (per-partition bcast)
    nslp_d = nc.dram_tensor("nslp", [128, NH], f32, kind="ExternalInput")
    y_d = nc.dram_tensor("ypart", [L, D], f32, kind="ExternalOutput")

    with tile.TileContext(nc) as tc:
        with tc.tile_pool(name="persist", bufs=1) as pp:
            # ---- persistent SBUF ----
            bqk_sb = pp.tile([128, NH], f32)
            nslp_sb = pp.tile([128, NH], f32)
            woutp_sb = pp.tile([128, 2, D], f32r)
            relb_sb = pp.tile([128, 4, 512], f32)
            ones_sb = pp.tile([1, 128], f32r)
            onesv_f = pp.tile([128, KT * NH], f32)
            V_sb = pp.tile([128, KT, NH, DH + 1], bf16)
            ctxA = pp.tile([128, L], f32r)  # heads 0,1 of group
            ctxB = pp.tile([64, L], f32r)  # head 2 of group
            # Per-head attention operand buffers.  The f32r PE rounds each
            # product to ~12 significand bits, so every aug product must be
            # a pre-rounded value times +-1: slope*idx terms are
            # premultiplied on the host (v/rv on the k side, rq/rr on the
            # q side, each split into a 12-sig-bit value plus residual).
            # Kbuf rows: 0-63 K^T, 64 mask, 65 v=rnd(s_r*k), 66 rv, 67 1, 68 1
            # QbufR rows: 0-63 Q'^T, 64 1, 65  1, 66  1, 67 -rq, 68 -rr
            # QbufL rows: 0-63 Q'^T, 64 1, 65 -1, 66 -1, 68  rq, 68  rr
            Kbuf = [
                pp.tile([69, L], f32r, tag=f"kb{j}", name=f"kb{j}") for j in range(NH)
            ]
            QbufR = [
                pp.tile([69, L], f32r, tag=f"qr{j}", name=f"qr{j}") for j in range(NH)
            ]
            QbufL = [
                pp.tile([69, L], f32r, tag=f"ql{j}", name=f"ql{j}") for j in range(NH)
            ]

            nc.sync.dma_start(bqk_sb[:], bqk_d[:])
            nc.sync.dma_start(nslp_sb[:], nslp_d[:])
            nc.sync.dma_start(woutp_sb[:], woutp_d.rearrange("(o p) n -> p o n", p=128))
            nc.sync.dma_start(relb_sb[:], relb_d[:])
            nc.sync.dma_start(ones_sb[:], ones_d[:])
            nc.gpsimd.memset(onesv_f[:], 1.0)
            # V softmax-denominator ones column (DVE copy rounds to f32r)
            nc.vector.tensor_copy(
                V_sb[:, :, :, DH : DH + 1].rearrange("p t h o -> p (t h o)"),
                onesv_f[:],
            )
            for j in range(NH):
                nc.sync.dma_start(QbufR[j][64:69, :], augqR_d[j])
                nc.sync.dma_start(QbufL[j][64:69, :], augqL_d[j])
                nc.sync.dma_start(Kbuf[j][64:69, :], augk_d[j])

            # Pre-touch DMA-loaded tiles consumed by TensorScalarPtr ops so
            # those ops carry a single sync wait (the walrus TS encoding
            # rejects multi-wait instructions).
            junk = pp.tile([1, 4], f32, name="junk")
            nc.vector.tensor_copy(junk[0:1, 0:1], bqk_sb[0:1, 0:1])
            nc.vector.tensor_copy(junk[0:1, 1:2], relb_sb[0:1, 0, 0:1])
            nc.vector.tensor_copy(junk[0:1, 2:3], nslp_sb[0:1, 0:1])

            # ---- stage 1: QKV projection ----
            with (
                tc.tile_pool(name="xpool", bufs=1) as xp,
                tc.tile_pool(name="ps1", bufs=4, space="PSUM") as ps1,
            ):
                wqk_sb = xp.tile([128, 6, 2 * DH * NH], f32r)
                wv_sb = xp.tile([128, 6, DH * NH], f32r)
                bv_sb = xp.tile([1, DH * NH], f32r)
                nc.sync.dma_start(wqk_sb[:], wqk_d.rearrange("(o p) m -> p o m", p=128))
                nc.sync.dma_start(wv_sb[:], wv_d.rearrange("(o p) m -> p o m", p=128))
                nc.sync.dma_start(bv_sb[:], bv_d[:])
                xT_sb = xp.tile([128, 6, L], f32r)
                for kt in range(6):
                    nc.sync.dma_start(
                        xT_sb[:, kt, :],
                        xT_d.rearrange("(o p) f -> p o f", p=128)[:, kt, :],
                    )

                # Q^T/K^T per head: PSUM [128, 512] = [Q^T_h; K^T_h] chunk
                for j in range(NH):
                    pcs = [
                        ps1.tile([128, 512], f32, tag="ps1", name=f"ps1c{c}")
                        for c in range(QC)
                    ]
                    for kt in range(6):
                        for c in range(QC):
                            nc.tensor.matmul(
                                pcs[c],
                                wqk_sb[:, kt, ts(j, 128)],
                                xT_sb[:, kt, ts(c, 512)],
                                start=(kt == 0),
                                stop=(kt == 5),
                            )
                    for c in range(QC):
                        ps = pcs[c]
                        cs = ts(c, 512)
                        nc.vector.tensor_scalar(
                            QbufR[j][0:64, cs],
                            ps[0:64, :],
                            0.125,
                            bqk_sb[0:64, j : j + 1],
                            MULT,
                            ADD,
                        )
                        nc.vector.tensor_copy(QbufL[j][0:64, cs], QbufR[j][0:64, cs])
                        nc.vector.tensor_scalar(
                            Kbuf[j][0:64, cs],
                            ps[64:128, :],
                            bqk_sb[64:128, j : j + 1],
                            None,
                            ADD,
                        )

                # V natural layout [l, d] + bias via K=1 matmul
                for lt in range(KT):
                    psv = ps1.tile([128, 512], f32, tag="ps1", name="psv")[:, : DH * NH]
                    for kt in range(6):
                        nc.tensor.matmul(
                            psv,
                            xT_sb[:, kt, ts(lt, 128)],
                            wv_sb[:, kt, :],
                            start=(kt == 0),
                            stop=False,
                        )
                    nc.tensor.matmul(
                        psv,
                        ones_sb[0:1, 0:128],
                        bv_sb[0:1, :],
                        start=False,
                        stop=True,
                    )
                    nc.scalar.copy(
                        V_sb[:, lt, :, 0:DH],
                        psv.rearrange("p (h x) -> p h x", x=DH),
                    )

            # ---- stage 2: attention ----
            # [1, 12*512] denom layout keeps every slice at partition base
            # 0 (activations reject input bases outside 0/32/64/96); the
            # pool wraps stages 2+2b and closes before stage 3.
            with tc.tile_pool(name="normp", bufs=1) as npool:
                denom_sb = npool.tile([1, NH * QC * 512], f32)
                lnr_sb = npool.tile([1, NH * QC * 512], f32)
                with (
                    tc.tile_pool(name="psS", bufs=3, space="PSUM") as psS,
                    tc.tile_pool(name="psO", bufs=2, space="PSUM") as psO,
                    tc.tile_pool(name="ptp", bufs=3) as ptp,
                ):

                    def emit_sgroup(j, c, t0, tn, st):
                        cs = ts(c, 512)
                        for i in range(tn):
                            t = t0 + i
                            js = ts(i, 512)
                            m = t - 4 * c
                            if 0 <= m < 4:  # diagonal block
                                nc.tensor.matmul(
                                    st[:, js],
                                    Kbuf[j][0:65, ts(t, 128)],
                                    QbufR[j][0:65, cs],
                                    start=True,
                                    stop=True,
                                )
                                nc.vector.scalar_tensor_tensor(
                                    st[:, js],
                                    relb_sb[:, m, :],
                                    nslp_sb[:, j : j + 1],
                                    st[:, js],
                                    MULT,
                                    ADD,
                                )
                            elif c > t // 4:  # keys before queries
                                nc.tensor.matmul(
                                    st[:, js],
                                    Kbuf[j][0:69, ts(t, 128)],
                                    QbufR[j][0:69, cs],
                                    start=True,
                                    stop=True,
                                )
                            else:  # keys after queries
                                nc.tensor.matmul(
                                    st[:, js],
                                    Kbuf[j][0:69, ts(t, 128)],
                                    QbufL[j][0:69, cs],
                                    start=True,
                                    stop=True,
                                )

                    def emit_exp_pv(j, c, t0, tn, st, out_t, t_lo, t_hi):
                        pt = ptp.tile(
                            [128, GROUP_SIZE * 512], bf16, tag="pt", name="pt"
                        )
                        nc.scalar.activation(pt[:, : tn * 512], st[:, : tn * 512], Exp)
                        for i in range(tn):
                            t = t0 + i
                            nc.tensor.matmul(
                                out_t[0 : DH + 1, :],
                                V_sb[:, t, j, :],
                                pt[:, ts(i, 512)],
                                start=(t == t_lo),
                                stop=(t == t_hi - 1),
                                skip_group_check=True,
                            )

                    def groups_of(j, c):
                        t_lo, t_hi = BANDS[j][c]
                        return [
                            (t0, min(GROUP_SIZE, t_hi - t0))
                            for t0 in range(t_lo, t_hi, GROUP_SIZE)
                        ]

                    # Software-pipeline each (head, chunk) pair: emit the
                    # S-matmuls one group ahead of exp/PV so the PE always
                    # has queued work while exp runs (a continuously-fed PE
                    # opens the HAM clock gate: 2.4GHz vs 1.2GHz).  Only
                    # one PSUM accumulation group (out_t) is open at a time
                    # (two open groups wedge the exec unit).
                    for j in range(NH):
                        for c in range(QC):
                            cs = ts(c, 512)
                            t_lo, t_hi = BANDS[j][c]
                            out_t = psO.tile(
                                [128, 512], f32, tag="outaug", name="outaug"
                            )
                            gl = groups_of(j, c)
                            sts = []
                            for g, (t0, tn) in enumerate(gl):
                                st = psS.tile(
                                    [128, GROUP_SIZE * 512], f32,
                                    tag="st", name="st",
                                )
                                sts.append(st)
                                emit_sgroup(j, c, t0, tn, st)
                                if g >= 1:
                                    emit_exp_pv(
                                        j, c, *gl[g - 1], sts[g - 1], out_t,
                                        t_lo, t_hi,
                                    )
                            emit_exp_pv(
                                j, c, *gl[-1], sts[-1], out_t, t_lo, t_hi
                            )
                            # stash unnormalized ctx + denom; normalize later
                            if j < 2:
                                ctx_slice = ctxA[j * 64 : (j + 1) * 64, cs]
                            else:
                                ctx_slice = ctxB[0:64, cs]
                            nc.scalar.copy(ctx_slice, out_t[0:DH, :])
                            nc.scalar.copy(
                                denom_sb[0:1, ts(j * QC + c, 512)],
                                out_t[DH : DH + 1, :],
                            )

                # ---- stage 2b: batched normalization ----
                # 1/denom = exp(-ln(denom)); one Ln batch then per-pair
                # Exp (table loads once each), GpSimd partition
                # broadcast, in-place DVE mul.
                with tc.tile_pool(name="recp", bufs=2) as rp:
                    nc.scalar.activation(lnr_sb[:], denom_sb[:], Ln)
                    # c-major so stage 3's first l-tiles (chunk 0) have
                    # their three heads normalized first
                    for c in range(QC):
                        for j in range(NH):
                            cs = ts(c, 512)
                            row = j * QC + c
                            # per-pair Exp lands at base partition 0 for
                            # the K=1 broadcast matmul (base 0/32/64)
                            rec = rp.tile([1, 512], f32, tag="rec")
                            nc.scalar.activation(
                                rec, lnr_sb[0:1, ts(row, 512)], Exp, scale=-1.0
                            )
                            recb = rp.tile([128, 512], f32, tag="recb")
                            nc.gpsimd.partition_broadcast(recb, rec)
                            # in-place mul needs equal SBUF base partitions
                            if j < 2:
                                ctx_slice = ctxA[j * 64 : (j + 1) * 64, cs]
                                recs = recb[j * 64 : (j + 1) * 64, :]
                            else:
                                ctx_slice = ctxB[0:64, cs]
                                recs = recb[0:64, :]
                            nc.vector.tensor_mul(ctx_slice, ctx_slice, recs)

            # ---- stage 3: output projection ----
            with (
                tc.tile_pool(name="ps3", bufs=2, space="PSUM") as ps3,
                tc.tile_pool(name="ysb", bufs=3) as yp,
            ):
                for lt in range(KT):
                    y = yp.tile([128, D], f32, tag="y")
                    for n0, nw in ((0, 512), (512, 256)):
                        ps = ps3.tile([128, 512], f32, tag="ps3", name="ps3t")[:, :nw]
                        nc.tensor.matmul(
                            ps,
                            ctxA[:, ts(lt, 128)],
                            woutp_sb[:, 0, n0 : n0 + nw],
                            start=True,
                            stop=False,
                        )
                        nc.tensor.matmul(
                            ps,
                            ctxB[0:64, ts(lt, 128)],
                            woutp_sb[0:64, 1, n0 : n0 + nw],
                            start=False,
                            stop=True,
                        )
                        nc.scalar.copy(y[:, n0 : n0 + nw], ps)
                    nc.sync.dma_start(y_d[ts(lt, 128), :], y)

    if not nc.is_finalized():
        nc.finalize()
    _PROGRAM_CACHE["nc"] = nc
    return nc


def _host_inputs(x, attn_mask, Wqkv, bqkv, Wout, bout):
    """Build the 8 per-core input dicts (f32r operands pre-rounded)."""
    slopes_r = _round10(alibi_slopes(H))
    x = np.asarray(x, dtype=np.float32)
    attn_mask = np.asarray(attn_mask)
    Wqkv = _round12(np.asarray(Wqkv, dtype=np.float32))
    bqkv = np.asarray(bqkv, dtype=np.float32)
    Wout = _round12(np.asarray(Wout, dtype=np.float32))
    bout = np.asarray(bout, dtype=np.float32)

    q_idx = np.arange(L, dtype=np.float32)
    ones_row = np.ones(L, dtype=np.float32)

    # rel_base[p, m, q'] = |q' - 128m - p|  (diagonal-block distance)
    p = np.arange(128, dtype=np.float32)[:, None, None]
    m = np.arange(4, dtype=np.float32)[None, :, None]
    qq = np.arange(512, dtype=np.float32)[None, None, :]
    relb = np.abs(qq - 128.0 * m - p).astype(np.float32)

    onesr = np.ones((1, 128), np.float32)

    in_maps = []
    for core in range(N_CORES):
        b = core // 4
        g = core % 4
        heads = HEAD_GROUPS[g]

        wqk = np.empty((D, 2 * DH * NH), np.float32)
        bqk = np.empty((128, NH), np.float32)
        wv = np.empty((D, DH * NH), np.float32)
        bv = np.empty((1, DH * NH), np.float32)
        woutp = np.zeros((256, D), np.float32)
        augk = np.empty((NH, 5, L), np.float32)
        augqR = np.empty((NH, 5, L), np.float32)
        augqL = np.empty((NH, 5, L), np.float32)
        nslp = np.empty((128, NH), np.float32)
        mask_row = np.where(attn_mask[b] == 0, NEG_MASK, 0.0).astype(np.float32)
        for jj, h in enumerate(heads):
            wqk[:, jj * 128 : jj * 128 + 64] = Wqkv[:, h * DH : (h + 1) * DH]
            wqk[:, jj * 128 + 64 : (jj + 1) * 128] = Wqkv[
                :, D + h * DH : D + (h + 1) * DH
            ]
            bqk[0:64, jj] = bqkv[h * DH : (h + 1) * DH] * 0.125
            bqk[64:128, jj] = bqkv[D + h * DH : D + (h + 1) * DH]
            wv[:, jj * DH : (jj + 1) * DH] = Wqkv[:, 2 * D + h * DH : 2 * D + (h + 1) * DH]
            bv[0, jj * DH : (jj + 1) * DH] = bqkv[2 * D + h * DH : 2 * D + (h + 1) * DH]
            woutp[jj * DH : (jj + 1) * DH, :] = Wout[h * DH : (h + 1) * DH, :]
            s = float(slopes_r[h])
            # s_r*idx premultiplied and split into a 12-significand-bit
            # value + residual (the f32r PE preserves x*1 only up to ~12
            # significand bits, so every aug row is (pre-rounded) * +-1)
            sk = np.float64(s) * q_idx.astype(np.float64)
            v = _round_mant(sk.astype(np.float32), 11)
            rv = _round_mant((sk - v.astype(np.float64)).astype(np.float32), 11)
            rq = v
            rr = rv
            augk[jj, 0, :] = _round_mant(mask_row, 11)
            augk[jj, 1, :] = v
            augk[jj, 2, :] = rv
            augk[jj, 3, :] = ones_row
            augk[jj, 4, :] = ones_row
            augqR[jj, 0, :] = ones_row
            augqR[jj, 1, :] = ones_row
            augqR[jj, 2, :] = ones_row
            augqR[jj, 3, :] = -rq
            augqR[jj, 4, :] = -rr
            augqL[jj, 0, :] = ones_row
            augqL[jj, 1, :] = -ones_row
            augqL[jj, 2, :] = -ones_row
            augqL[jj, 3, :] = rq
            augqL[jj, 4, :] = rr
            nslp[:, jj] = -s
        in_maps.append(
            {
                "xT": _round12(np.ascontiguousarray(x[b].T)),
                "wqk": wqk,
                "bqk": bqk,
                "wv": _round12(wv),
                "bv": _round12(bv),
                "woutp": woutp,
                "onesr": onesr,
                "augqR": augqR.copy(),
                "augqL": augqL.copy(),
                "augk": augk.copy(),
                "relb": relb,
                "nslp": nslp,
            }
        )
    return in_maps


def kernel(x, attn_mask, Wqkv, bqkv, Wout, bout):
    _ensure_concourse()
    from concourse.bass_utils import run_bass_kernel_spmd

    nc = _build_program()
    in_maps = _host_inputs(x, attn_mask, Wqkv, bqkv, Wout, bout)

    res = run_bass_kernel_spmd(
        nc,
        in_maps,
        list(range(N_CORES)),
        trace=bool(os.environ.get("BASS_TRACE")),
    )
    outs = [r["ypart"] for r in res.results]
    out = np.zeros((B, L, D), np.float32)
    for core in range(N_CORES):
        out[core // 4] += outs[core]
    out += np.asarray(bout, np.float32)[None, None, :]
    kernel.last_result = res
    if res.exec_time_ns is not None:
        kernel.last_exec_time_ns = res.exec_time_ns
    return out


# revision 34
# speedup vs baseline: 2.5212x; 1.0223x over previous
"""MultiHeadSelfAttention + ALiBi for Trainium2, SPMD over 8 NeuronCores.

Sharding: core c handles batch b = c // 4 and head group g = c % 4
(3 of the 12 heads, grouped so per-head ALiBi band sizes balance).
Each core computes y_partial[b] = ctx(heads_g) @ Wout[rows_g]; the host
sums the 4 partials per batch and adds bout.

All matmuls run in float32r (12-mantissa-bit fast mode, 4x the fp32
rate at free-dim >= 256).  f32r matmul operands must come from rounding
producers; DMA qualifies when the DRAM tensor is declared f32r, so the
host pre-rounds x/weights/aug rows to the 12-bit grid and the kernel
DMAs them straight into f32r tiles.  Device-computed operands (Q/K/V,
P=exp(S), ctx) round via their producing DVE/ACT instruction.

ALiBi handling under reduced precision: the per-head slope is
pre-rounded to 10 mantissa bits (s_r) and used consistently on both the
k and q aug rows, making slope rounding a per-head slope perturbation
(rel ~5e-4, harmless) instead of a softmax distortion.  The s_r*k_idx
aug row splits into v = round12(s_r*k) plus a residual row
r = s_r*k - v so large magnitudes survive the 12-bit grid exactly.

Device pipeline per core:
  1. QK^T = Wqk^T @ x^T  -> per head: Q'/8+bq into dual Q buffers, K+bk
     into K buffer.  V = x @ Wv + bv (ones column appended per head for
     softmax denominators).
  2. S^T blocks [128k x 512q]: matmul with augmented contraction rows
     carrying the attention mask bias and, off-diagonal, the ALiBi term
     -s_r*|q-k| (linear there).  Diagonal blocks get a fused DVE
     (rel_base * -s_r + S) pass.  exp() on ScalarE over 3-block groups,
     P^T @ V_aug accumulated in PSUM -> unnormalized ctx^T + denom row,
     copied to SBUF per (head, chunk).
  3. Batched normalization: one Ln over all 12 denom rows, then per-pair
     Exp(-x) (exp table loads once), K=1 broadcast matmul + in-place DVE
     multiply.  1/x = exp(-ln x) because the DVE reciprocal ops
     misbehave under this runtime.
  4. y = ctx^T.T @ Wout rows.  Blocks where ALiBi decays attention below
     ~2e-4 relative are skipped per the BANDS table (bout on the host).
"""

import math
import os

import numpy as np


def _ensure_concourse():
    try:
        import concourse  # noqa: F401
    except ImportError:
        import sys

        for p in ("/opt/trn_rl_repo", "/root/.axon_site/_ro/trn_rl_repo"):
            if os.path.isdir(p) and p not in sys.path:
                sys.path.insert(0, p)


B, L, D, H, DH = 2, 2048, 768, 12, 64
KT = L // 128  # 16 k-tiles
QC = L // 512  # 4 q-chunks
NH = 3  # heads per core
N_CORES = 8
GROUP_SIZE = 2  # exp/S group size in k-tiles (2 PSUM banks)

# Per head-slot key-tile bands per q-chunk (t_lo, t_hi_exclusive).  Slot
# 0 holds the wide-band heads (full attention); slots 1/2 hold heads
# whose ALiBi decays attention to ~exp(-36) beyond d_max = 36/slope
# keys.  The margin must cover worst-case Q.K swings (~±5 on each side
# of the exponent), so excluded blocks leak < ~2048*e^(-26) ~ 1e-8.
BANDS = [
    [(0, 16), (0, 16), (0, 16), (0, 16)],  # slot 0: full
    [(0, 9), (0, 13), (3, 16), (7, 16)],  # slot 1: d=576
    [(0, 6), (2, 10), (6, 14), (10, 16)],  # slot 2: d=204
]

# Head groups balanced by ALiBi band size (slopes below): each group gets
# one wide-band, one mid-band and one narrow-band head.
HEAD_GROUPS = [[4, 3, 0], [5, 2, 8], [6, 11, 9], [7, 1, 10]]

NEG_MASK = -1.0e9


def alibi_slopes(n_heads: int) -> np.ndarray:
    def slopes_pow2(n):
        start = 2 ** (-(2 ** -(math.log2(n) - 3)))
        return [start * start**i for i in range(n)]

    if math.log2(n_heads).is_integer():
        s = slopes_pow2(n_heads)
    else:
        cp = 2 ** int(math.floor(math.log2(n_heads)))
        s = slopes_pow2(cp) + slopes_pow2(2 * cp)[0::2][: n_heads - cp]
    return np.asarray(s, dtype=np.float32)


def _round_mant(x, bits):
    """Round fp32 values to `bits` explicit mantissa bits (RNE), i.e. onto
    the f32r grid (12) or safely within it (10)."""
    x = np.asarray(x, np.float32)
    b = x.view(np.uint32).copy()
    drop = 23 - bits
    b = b + (((b >> drop) & 1) + np.uint32((1 << (drop - 1)) - 1))
    b &= np.uint32(~((1 << drop) - 1) & 0xFFFFFFFF)
    return b.view(np.float32)


def _round10(x):
    return _round_mant(x, 10)


def _round12(x):
    return _round_mant(x, 12)


_PROGRAM_CACHE = {}


def _build_program():
    """Build the (shared, SPMD) Bass program once."""
    if "nc" in _PROGRAM_CACHE:
        return _PROGRAM_CACHE["nc"]

    _ensure_concourse()
    import concourse.mybir as mybir
    import concourse.tile as tile
    from concourse import bacc
    from concourse.bass import ts

    f32 = mybir.dt.float32
    f32r = mybir.dt.float32r
    bf16 = mybir.dt.bfloat16
    Exp = mybir.ActivationFunctionType.Exp
    Ln = mybir.ActivationFunctionType.Ln
    MULT = mybir.AluOpType.mult
    ADD = mybir.AluOpType.add

    nc = bacc.Bacc(None)

    # ---- DRAM I/O (f32r tensors arrive pre-rounded from the host) ----
    xT_d = nc.dram_tensor("xT", [D, L], f32r, kind="ExternalInput")
    wqk_d = nc.dram_tensor("wqk", [D, 2 * DH * NH], f32r, kind="ExternalInput")
    bqk_d = nc.dram_tensor("bqk", [128, NH], f32, kind="ExternalInput")
    wv_d = nc.dram_tensor("wv", [D, DH * NH], f32r, kind="ExternalInput")
    bv_d = nc.dram_tensor("bv", [1, DH * NH], f32r, kind="ExternalInput")
    woutp_d = nc.dram_tensor("woutp", [256, D], f32r, kind="ExternalInput")
    ones_d = nc.dram_tensor("onesr", [1, 128], f32r, kind="ExternalInput")
    augqR_d = nc.dram_tensor("augqR", [NH, 5, L], f32r, kind="ExternalInput")
    augqL_d = nc.dram_tensor("augqL", [NH, 5, L], f32r, kind="ExternalInput")
    augk_d = nc.dram_tensor("augk", [NH, 5, L], f32r, kind="ExternalInput")
    # rel_base[p, m, q'] = |q' - 128m - p| (unscaled; -s_r applied on DVE)
    relb_d = nc.dram_tensor("relb", [128, 4, 512], f32, kind="ExternalInput")
    # negslope[p, j] = -s_r of the core's head slot j (per-partition bcast)
    nslp_d = nc.dram_tensor("nslp", [128, NH], f32, kind="ExternalInput")
    y_d = nc.dram_tensor("ypart", [L, D], f32, kind="ExternalOutput")

    with tile.TileContext(nc) as tc:
        with tc.tile_pool(name="persist", bufs=1) as pp:
            # ---- persistent SBUF ----
            bqk_sb = pp.tile([128, NH], f32)
            nslp_sb = pp.tile([128, NH], f32)
            woutp_sb = pp.tile([128, 2, D], f32r)
            relb_sb = pp.tile([128, 4, 512], f32)
            ones_sb = pp.tile([1, 128], f32r)
            onesv_f = pp.tile([128, KT * NH], f32)
            V_sb = pp.tile([128, KT, NH, DH + 1], bf16)
            ctxA = pp.tile([128, L], f32r)  # heads 0,1 of group
            ctxB = pp.tile([64, L], f32r)  # head 2 of group
            # Per-head attention operand buffers.  The f32r PE rounds each
            # product to ~12 significand bits, so every aug product must be
            # a pre-rounded value times +-1: slope*idx terms are
            # premultiplied on the host (v/rv on the k side, rq/rr on the
            # q side, each split into a 12-sig-bit value plus residual).
            # Kbuf rows: 0-63 K^T, 64 mask, 65 v=rnd(s_r*k), 66 rv, 67 1, 68 1
            # QbufR rows: 0-63 Q'^T, 64 1, 65  1, 66  1, 67 -rq, 68 -rr
            # QbufL rows: 0-63 Q'^T, 64 1, 65 -1, 66 -1, 68  rq, 68  rr
            Kbuf = [
                pp.tile([69, L], f32r, tag=f"kb{j}", name=f"kb{j}") for j in range(NH)
            ]
            QbufR = [
                pp.tile([69, L], f32r, tag=f"qr{j}", name=f"qr{j}") for j in range(NH)
            ]
            QbufL = [
                pp.tile([69, L], f32r, tag=f"ql{j}", name=f"ql{j}") for j in range(NH)
            ]

            nc.sync.dma_start(bqk_sb[:], bqk_d[:])
            nc.sync.dma_start(nslp_sb[:], nslp_d[:])
            nc.sync.dma_start(ones_sb[:], ones_d[:])
            nc.gpsimd.memset(onesv_f[:], 1.0)
            # V softmax-denominator ones column (DVE copy rounds to f32r)
            nc.vector.tensor_copy(
                V_sb[:, :, :, DH : DH + 1].rearrange("p t h o -> p (t h o)"),
                onesv_f[:],
            )

            # ---- stage 1: QKV projection ----
            with (
                tc.tile_pool(name="xpool", bufs=1) as xp,
                tc.tile_pool(name="ps1", bufs=4, space="PSUM") as ps1,
            ):
                wqk_sb = xp.tile([128, 6, 2 * DH * NH], f32r)
                wv_sb = xp.tile([128, 6, DH * NH], f32r)
                bv_sb = xp.tile([1, DH * NH], f32r)
                # stage-1-critical DMAs first; stage-2/3 operands (aug
                # rows, relb, woutp) queue behind them
                nc.sync.dma_start(wqk_sb[:], wqk_d.rearrange("(o p) m -> p o m", p=128))
                nc.sync.dma_start(wv_sb[:], wv_d.rearrange("(o p) m -> p o m", p=128))
                nc.sync.dma_start(bv_sb[:], bv_d[:])
                xT_sb = xp.tile([128, 6, L], f32r)
                for kt in range(6):
                    nc.sync.dma_start(
                        xT_sb[:, kt, :],
                        xT_d.rearrange("(o p) f -> p o f", p=128)[:, kt, :],
                    )
                for j in range(NH):
                    nc.sync.dma_start(QbufR[j][64:69, :], augqR_d[j])
                    nc.sync.dma_start(QbufL[j][64:69, :], augqL_d[j])
                    nc.sync.dma_start(Kbuf[j][64:69, :], augk_d[j])
                nc.sync.dma_start(relb_sb[:], relb_d[:])
                nc.sync.dma_start(
                    woutp_sb[:], woutp_d.rearrange("(o p) n -> p o n", p=128)
                )

                # Pre-touch DMA-loaded tiles consumed by TensorScalarPtr
                # ops so those ops carry a single sync wait (the walrus TS
                # encoding rejects multi-wait instructions).
                junk = pp.tile([1, 4], f32, name="junk")
                nc.vector.tensor_copy(junk[0:1, 0:1], bqk_sb[0:1, 0:1])
                nc.vector.tensor_copy(junk[0:1, 1:2], relb_sb[0:1, 0, 0:1])
                nc.vector.tensor_copy(junk[0:1, 2:3], nslp_sb[0:1, 0:1])

                # Q^T/K^T per head: PSUM [128, 512] = [Q^T_h; K^T_h] chunk
                for j in range(NH):
                    pcs = [
                        ps1.tile([128, 512], f32, tag="ps1", name=f"ps1c{c}")
                        for c in range(QC)
                    ]
                    for kt in range(6):
                        for c in range(QC):
                            nc.tensor.matmul(
                                pcs[c],
                                wqk_sb[:, kt, ts(j, 128)],
                                xT_sb[:, kt, ts(c, 512)],
                                start=(kt == 0),
                                stop=(kt == 5),
                            )
                    for c in range(QC):
                        ps = pcs[c]
                        cs = ts(c, 512)
                        nc.vector.tensor_scalar(
                            QbufR[j][0:64, cs],
                            ps[0:64, :],
                            0.125,
                            bqk_sb[0:64, j : j + 1],
                            MULT,
                            ADD,
                        )
                        nc.vector.tensor_copy(QbufL[j][0:64, cs], QbufR[j][0:64, cs])
                        nc.vector.tensor_scalar(
                            Kbuf[j][0:64, cs],
                            ps[64:128, :],
                            bqk_sb[64:128, j : j + 1],
                            None,
                            ADD,
                        )

                # V natural layout [l, d] + bias via K=1 matmul
                for lt in range(KT):
                    psv = ps1.tile([128, 512], f32, tag="ps1", name="psv")[:, : DH * NH]
                    for kt in range(6):
                        nc.tensor.matmul(
                            psv,
                            xT_sb[:, kt, ts(lt, 128)],
                            wv_sb[:, kt, :],
                            start=(kt == 0),
                            stop=False,
                        )
                    nc.tensor.matmul(
                        psv,
                        ones_sb[0:1, 0:128],
                        bv_sb[0:1, :],
                        start=False,
                        stop=True,
                    )
                    nc.scalar.copy(
                        V_sb[:, lt, :, 0:DH],
                        psv.rearrange("p (h x) -> p h x", x=DH),
                    )

            # ---- stage 2: attention ----
            # [1, 12*512] denom layout keeps every slice at partition base
            # 0 (activations reject input bases outside 0/32/64/96); the
            # pool wraps stages 2+2b and closes before stage 3.
            with tc.tile_pool(name="normp", bufs=1) as npool:
                denom_sb = npool.tile([1, NH * QC * 512], f32)
                lnr_sb = npool.tile([1, NH * QC * 512], f32)
                with (
                    tc.tile_pool(name="psS", bufs=3, space="PSUM") as psS,
                    tc.tile_pool(name="psO", bufs=2, space="PSUM") as psO,
                    tc.tile_pool(name="ptp", bufs=3) as ptp,
                ):

                    def emit_sgroup(j, c, t0, tn, st):
                        cs = ts(c, 512)
                        for i in range(tn):
                            t = t0 + i
                            js = ts(i, 512)
                            m = t - 4 * c
                            if 0 <= m < 4:  # diagonal block
                                nc.tensor.matmul(
                                    st[:, js],
                                    Kbuf[j][0:65, ts(t, 128)],
                                    QbufR[j][0:65, cs],
                                    start=True,
                                    stop=True,
                                )
                                nc.vector.scalar_tensor_tensor(
                                    st[:, js],
                                    relb_sb[:, m, :],
                                    nslp_sb[:, j : j + 1],
                                    st[:, js],
                                    MULT,
                                    ADD,
                                )
                            elif c > t // 4:  # keys before queries
                                nc.tensor.matmul(
                                    st[:, js],
                                    Kbuf[j][0:69, ts(t, 128)],
                                    QbufR[j][0:69, cs],
                                    start=True,
                                    stop=True,
                                )
                            else:  # keys after queries
                                nc.tensor.matmul(
                                    st[:, js],
                                    Kbuf[j][0:69, ts(t, 128)],
                                    QbufL[j][0:69, cs],
                                    start=True,
                                    stop=True,
                                )

                    def emit_exp_pv(j, c, t0, tn, st, out_t, t_lo, t_hi):
                        pt = ptp.tile(
                            [128, GROUP_SIZE * 512], bf16, tag="pt", name="pt"
                        )
                        nc.scalar.activation(pt[:, : tn * 512], st[:, : tn * 512], Exp)
                        for i in range(tn):
                            t = t0 + i
                            nc.tensor.matmul(
                                out_t[0 : DH + 1, :],
                                V_sb[:, t, j, :],
                                pt[:, ts(i, 512)],
                                start=(t == t_lo),
                                stop=(t == t_hi - 1),
                                skip_group_check=True,
                            )

                    def groups_of(j, c):
                        t_lo, t_hi = BANDS[j][c]
                        return [
                            (t0, min(GROUP_SIZE, t_hi - t0))
                            for t0 in range(t_lo, t_hi, GROUP_SIZE)
                        ]

                    # Software-pipeline each (head, chunk) pair: emit the
                    # S-matmuls one group ahead of exp/PV so the PE always
                    # has queued work while exp runs (a continuously-fed PE
                    # opens the HAM clock gate: 2.4GHz vs 1.2GHz).  Only
                    # one PSUM accumulation group (out_t) is open at a time
                    # (two open groups wedge the exec unit).
                    for j in range(NH):
                        for c in range(QC):
                            cs = ts(c, 512)
                            t_lo, t_hi = BANDS[j][c]
                            out_t = psO.tile(
                                [128, 512], f32, tag="outaug", name="outaug"
                            )
                            gl = groups_of(j, c)
                            sts = []
                            for g, (t0, tn) in enumerate(gl):
                                st = psS.tile(
                                    [128, GROUP_SIZE * 512], f32,
                                    tag="st", name="st",
                                )
                                sts.append(st)
                                emit_sgroup(j, c, t0, tn, st)
                                if g >= 1:
                                    emit_exp_pv(
                                        j, c, *gl[g - 1], sts[g - 1], out_t,
                                        t_lo, t_hi,
                                    )
                            emit_exp_pv(
                                j, c, *gl[-1], sts[-1], out_t, t_lo, t_hi
                            )
                            # stash unnormalized ctx + denom; normalize later
                            if j < 2:
                                ctx_slice = ctxA[j * 64 : (j + 1) * 64, cs]
                            else:
                                ctx_slice = ctxB[0:64, cs]
                            nc.scalar.copy(ctx_slice, out_t[0:DH, :])
                            nc.scalar.copy(
                                denom_sb[0:1, ts(j * QC + c, 512)],
                                out_t[DH : DH + 1, :],
                            )

                # ---- stage 2b: batched normalization ----
                # 1/denom = exp(-ln(denom)); one Ln batch then per-pair
                # Exp (table loads once each), GpSimd partition
                # broadcast, in-place DVE mul.
                with tc.tile_pool(name="recp", bufs=2) as rp:
                    nc.scalar.activation(lnr_sb[:], denom_sb[:], Ln)
                    # c-major so stage 3's first l-tiles (chunk 0) have
                    # their three heads normalized first
                    for c in range(QC):
                        for j in range(NH):
                            cs = ts(c, 512)
                            row = j * QC + c
                            # per-pair Exp lands at base partition 0 for
                            # the K=1 broadcast matmul (base 0/32/64)
                            rec = rp.tile([1, 512], f32, tag="rec")
                            nc.scalar.activation(
                                rec, lnr_sb[0:1, ts(row, 512)], Exp, scale=-1.0
                            )
                            recb = rp.tile([128, 512], f32, tag="recb")
                            nc.gpsimd.partition_broadcast(recb, rec)
                            # in-place mul needs equal SBUF base partitions
                            if j < 2:
                                ctx_slice = ctxA[j * 64 : (j + 1) * 64, cs]
                                recs = recb[j * 64 : (j + 1) * 64, :]
                            else:
                                ctx_slice = ctxB[0:64, cs]
                                recs = recb[0:64, :]
                            nc.vector.tensor_mul(ctx_slice, ctx_slice, recs)

            # ---- stage 3: output projection ----
            with (
                tc.tile_pool(name="ps3", bufs=2, space="PSUM") as ps3,
                tc.tile_pool(name="ysb", bufs=3) as yp,
            ):
                for lt in range(KT):
                    y = yp.tile([128, D], f32, tag="y")
                    for n0, nw in ((0, 512), (512, 256)):
                        ps = ps3.tile([128, 512], f32, tag="ps3", name="ps3t")[:, :nw]
                        nc.tensor.matmul(
                            ps,
                            ctxA[:, ts(lt, 128)],
                            woutp_sb[:, 0, n0 : n0 + nw],
                            start=True,
                            stop=False,
                        )
                        nc.tensor.matmul(
                            ps,
                            ctxB[0:64, ts(lt, 128)],
                            woutp_sb[0:64, 1, n0 : n0 + nw],
                            start=False,
                            stop=True,
                        )
                        nc.scalar.copy(y[:, n0 : n0 + nw], ps)
                    nc.sync.dma_start(y_d[ts(lt, 128), :], y)

    if not nc.is_finalized():
        nc.finalize()
    _PROGRAM_CACHE["nc"] = nc
    return nc


def _host_inputs(x, attn_mask, Wqkv, bqkv, Wout, bout):
    """Build the 8 per-core input dicts (f32r operands pre-rounded)."""
    slopes_r = _round10(alibi_slopes(H))
    x = np.asarray(x, dtype=np.float32)
    attn_mask = np.asarray(attn_mask)
    Wqkv = _round12(np.asarray(Wqkv, dtype=np.float32))
    bqkv = np.asarray(bqkv, dtype=np.float32)
    Wout = _round12(np.asarray(Wout, dtype=np.float32))
    bout = np.asarray(bout, dtype=np.float32)

    q_idx = np.arange(L, dtype=np.float32)
    ones_row = np.ones(L, dtype=np.float32)

    # rel_base[p, m, q'] = |q' - 128m - p|  (diagonal-block distance)
    p = np.arange(128, dtype=np.float32)[:, None, None]
    m = np.arange(4, dtype=np.float32)[None, :, None]
    qq = np.arange(512, dtype=np.float32)[None, None, :]
    relb = np.abs(qq - 128.0 * m - p).astype(np.float32)

    onesr = np.ones((1, 128), np.float32)

    in_maps = []
    for core in range(N_CORES):
        b = core // 4
        g = core % 4
        heads = HEAD_GROUPS[g]

        wqk = np.empty((D, 2 * DH * NH), np.float32)
        bqk = np.empty((128, NH), np.float32)
        wv = np.empty((D, DH * NH), np.float32)
        bv = np.empty((1, DH * NH), np.float32)
        woutp = np.zeros((256, D), np.float32)
        augk = np.empty((NH, 5, L), np.float32)
        augqR = np.empty((NH, 5, L), np.float32)
        augqL = np.empty((NH, 5, L), np.float32)
        nslp = np.empty((128, NH), np.float32)
        mask_row = np.where(attn_mask[b] == 0, NEG_MASK, 0.0).astype(np.float32)
        for jj, h in enumerate(heads):
            wqk[:, jj * 128 : jj * 128 + 64] = Wqkv[:, h * DH : (h + 1) * DH]
            wqk[:, jj * 128 + 64 : (jj + 1) * 128] = Wqkv[
                :, D + h * DH : D + (h + 1) * DH
            ]
            bqk[0:64, jj] = bqkv[h * DH : (h + 1) * DH] * 0.125
            bqk[64:128, jj] = bqkv[D + h * DH : D + (h + 1) * DH]
            wv[:, jj * DH : (jj + 1) * DH] = Wqkv[:, 2 * D + h * DH : 2 * D + (h + 1) * DH]
            bv[0, jj * DH : (jj + 1) * DH] = bqkv[2 * D + h * DH : 2 * D + (h + 1) * DH]
            woutp[jj * DH : (jj + 1) * DH, :] = Wout[h * DH : (h + 1) * DH, :]
            s = float(slopes_r[h])
            # s_r*idx premultiplied and split into a 12-significand-bit
            # value + residual (the f32r PE preserves x*1 only up to ~12
            # significand bits, so every aug row is (pre-rounded) * +-1)
            sk = np.float64(s) * q_idx.astype(np.float64)
            v = _round_mant(sk.astype(np.float32), 11)
            rv = _round_mant((sk - v.astype(np.float64)).astype(np.float32), 11)
            rq = v
            rr = rv
            augk[jj, 0, :] = _round_mant(mask_row, 11)
            augk[jj, 1, :] = v
            augk[jj, 2, :] = rv
            augk[jj, 3, :] = ones_row
            augk[jj, 4, :] = ones_row
            augqR[jj, 0, :] = ones_row
            augqR[jj, 1, :] = ones_row
            augqR[jj, 2, :] = ones_row
            augqR[jj, 3, :] = -rq
            augqR[jj, 4, :] = -rr
            augqL[jj, 0, :] = ones_row
            augqL[jj, 1, :] = -ones_row
            augqL[jj, 2, :] = -ones_row
            augqL[jj, 3, :] = rq
            augqL[jj, 4, :] = rr
            nslp[:, jj] = -s
        in_maps.append(
            {
                "xT": _round12(np.ascontiguousarray(x[b].T)),
                "wqk": wqk,
                "bqk": bqk,
                "wv": _round12(wv),
                "bv": _round12(bv),
                "woutp": woutp,
                "onesr": onesr,
                "augqR": augqR.copy(),
                "augqL": augqL.copy(),
                "augk": augk.copy(),
                "relb": relb,
                "nslp": nslp,
            }
        )
    return in_maps


def kernel(x, attn_mask, Wqkv, bqkv, Wout, bout):
    _ensure_concourse()
    from concourse.bass_utils import run_bass_kernel_spmd

    nc = _build_program()
    in_maps = _host_inputs(x, attn_mask, Wqkv, bqkv, Wout, bout)

    res = run_bass_kernel_spmd(
        nc,
        in_maps,
        list(range(N_CORES)),
        trace=bool(os.environ.get("BASS_TRACE")),
    )
    outs = [r["ypart"] for r in res.results]
    out = np.zeros((B, L, D), np.float32)
    for core in range(N_CORES):
        out[core // 4] += outs[core]
    out += np.asarray(bout, np.float32)[None, None, :]
    kernel.last_result = res
    if res.exec_time_ns is not None:
        kernel.last_exec_time_ns = res.exec_time_ns
    return out
